# revision 16
# baseline (speedup 1.0000x reference)
"""Trainium2 Bass kernel for nn_DiffHistogram (Gaussian soft-binned histogram).

out[b, c*32+k, 0, 0] = sum_{h,w} (ER/RATIO) * exp(-(x-c_k)^2 / (2*sigma^2))
x: [8, 8, 256, 256] f32, bin centers equally spaced on [0, 1].

Sharding: data-parallel over batch B across 8 NeuronCores; per-core SBUF
layout [128, 4096], partition p = c*16+g.

Algorithm (anchor + geometric chain, multi-engine):
  Equal bin spacing D gives w_{k+1}(x) = w_k(x)*u(x) with
  u = exp(2*a*D*(x-gamma)) up to a known per-bin constant (host-folded).
  - ACT: u via Exp (2 half passes), anchor/direct bins via Derivative_Erf
    (accum_out = that bin's per-partition sums for free).
  - DVE (+ optionally GPSIMD): one bf16 tensor_tensor multiply per chain
    bin (~1.8us at 2x mode). Chains split into odd/even subchains via
    U2 = u^2.
  - PE: block-ones matmul reduces each chain tile into a PSUM slot
    (32 chunk matmuls, ~1.7us/bin); the [8, 128] residuals are DMA'd to
    HBM at the end and summed on the host.

HW-CRITICAL: on this hardware a DVE/GP/PE `wait_ge` costs ~1.6us EVEN IF
ALREADY SATISFIED (measured: bf16 TT 1813ns -> 6567ns with +3 waits).
So the HW build ("lean" mode) emits only the cross-engine waits required
for correctness, batched where possible. CoreSim's race detector needs
edges for every hazard incl. same-engine, so sim builds (DIFFHIST_FULLSEMS
=1, set by test.py) add the always-true self-edges back.

Numerics: bf16 chains + f32 accumulation, rel err ~1.7e-4 (tol 2e-2).
"""

import contextlib
import math
import os

import numpy as np

import concourse.bass as bass
import concourse.mybir as mybir
from concourse.bass_utils import run_bass_kernel_spmd

B = 8
C = 8
HW = 256 * 256
NBINS = 32
G = 128 // C
FREE = HW // G          # 4096

ER = 1.0
RATIO = 2.5066
SIGMA = 1.0 / NBINS
A_COEF = 1.0 / (2.0 * SIGMA * SIGMA)       # 512.0
SQRT_A = math.sqrt(A_COEF)                 # 22.627417
GAMMA = 0.5

FULLSEMS = os.environ.get("DIFFHIST_FULLSEMS", "0") == "1"

_CFG = os.environ.get("DIFFHIST_CFG", "v1")
if _CFG == "v1":
    SEGMENTS = [
        (12, [(1, "D"), (2, "G"), (3, "D"), (4, "G"), (5, "D"), (6, "G"), (7, "D")]),
        (4,  [(1, "D"), (2, "G"), (3, "D"), (4, "G"), (5, "D"), (6, "G"), (7, "D")]),
        (22, [(1, "D"), (2, "G"), (3, "D"), (4, "G")]),
        (0,  [(1, "D"), (2, "G"), (3, "D")]),
        (29, [(1, "D"), (2, "D")]),
    ]
    DIRECT = [20, 21, 27, 28]
elif _CFG == "nogp":
    SEGMENTS = [
        (12, [(1, "D"), (2, "D"), (3, "D"), (4, "D"), (5, "D"), (6, "D"), (7, "D")]),
        (4,  [(1, "D"), (2, "D"), (3, "D"), (4, "D"), (5, "D"), (6, "D"), (7, "D")]),
        (22, [(1, "D"), (2, "D"), (3, "D"), (4, "D")]),
        (0,  [(1, "D"), (2, "D"), (3, "D")]),
        (29, [(1, "D"), (2, "D")]),
    ]
    DIRECT = [20, 21, 27, 28]
elif _CFG == "gp6":
    SEGMENTS = [
        (12, [(1, "D"), (2, "G"), (3, "D"), (4, "G"), (5, "D"), (6, "D"), (7, "D")]),
        (4,  [(1, "D"), (2, "G"), (3, "D"), (4, "G"), (5, "D"), (6, "D"), (7, "D")]),
        (22, [(1, "D"), (2, "G"), (3, "D"), (4, "D")]),
        (0,  [(1, "D"), (2, "G"), (3, "D")]),
        (29, [(1, "D"), (2, "D")]),
    ]
    DIRECT = [20, 21, 27, 28]
else:
    raise ValueError(_CFG)

ND = 6
NG = 4
NA = 4
RING_BATCH_D = 3
RING_BATCH_G = 2
PE_BATCH = 2

COST_D = 1900.0
COST_G = 3500.0
COST_PE = 1750.0
COST_ACT = 3900.0

_nc_cache: dict = {}
last_results = None


def _plan():
    tiles = []
    nd = ng = 0
    for si, (k0, chain) in enumerate(SEGMENTS):
        for (j, eng) in chain:
            t = {"bin": k0 + j, "seg": si, "j": j, "eng": eng}
            if eng == "D":
                t["local"] = nd
                nd += 1
            else:
                t["local"] = ng
                ng += 1
            tiles.append(t)
    n_d, n_g = nd, ng
    d_tiles = [t for t in tiles if t["eng"] == "D"]
    g_tiles = [t for t in tiles if t["eng"] == "G"]

    # estimated completion times, for PE consumption order only
    t_u = 7400.0 + 2 * 1200.0
    anchor_done = {}
    tact = t_u + 1300.0
    for si in range(len(SEGMENTS)):
        tact += COST_ACT
        anchor_done[si] = tact
    tile_done = {}
    tdve = t_u + COST_D
    for t in d_tiles:
        dep = anchor_done[t["seg"]] if t["j"] <= 2 else tile_done[(t["seg"], t["j"] - 2)]
        tdve = max(tdve, dep) + COST_D
        tile_done[(t["seg"], t["j"])] = tdve
        t["est"] = tdve
    tgp = t_u
    for t in g_tiles:
        dep = anchor_done[t["seg"]] if t["j"] <= 2 else tile_done[(t["seg"], t["j"] - 2)]
        tgp = max(tgp, dep) + COST_G
        tile_done[(t["seg"], t["j"])] = tgp
        t["est"] = tgp

    pe_order = sorted(tiles, key=lambda t: t["est"])
    cd = cg = 0
    for slot, t in enumerate(pe_order):
        t["slot"] = slot
        if t["eng"] == "D":
            cd += 1
            t["pe_cum"] = cd
        else:
            cg += 1
            t["pe_cum"] = cg
    return tiles, d_tiles, g_tiles, pe_order, n_d, n_g


def _drift(bc: np.ndarray):
    bc = np.asarray(bc, np.float64)
    delta = (bc[-1] - bc[0]) / (NBINS - 1)
    su = 2.0 * A_COEF * delta
    bu = -su * GAMMA
    out = {}
    for k0, chain in SEGMENTS:
        c0 = bc[k0]
        for (j, _e) in chain:
            out[k0 + j] = A_COEF * ((c0 + j * delta) ** 2 - c0 ** 2) + j * bu
    return out, su, bu, delta


def _build(bin_centers: np.ndarray, reps: int = 1) -> "bass.Bass":
    bc = np.asarray(bin_centers, np.float64)
    nodma = os.environ.get("DIFFHIST_NODMA", "0") == "1"
    key = (reps, nodma, FULLSEMS, _CFG, tuple(bc.tolist()))
    if key in _nc_cache:
        return _nc_cache[key]

    tiles, d_tiles, g_tiles, pe_order, n_d, n_g = _plan()
    n_tiles = len(tiles)
    n_seg = len(SEGMENTS)
    n_act = n_seg + len(DIRECT)
    _dr, su, bu, delta = _drift(bc)

    f32 = mybir.dt.float32
    bf16 = mybir.dt.bfloat16
    alu = mybir.AluOpType
    act_fn = mybir.ActivationFunctionType

    last_u_d = max([i for i, t in enumerate(d_tiles) if t["j"] == 1], default=-1)
    last_u_g = max([i for i, t in enumerate(g_tiles) if t["j"] == 1], default=-1)
    last_u2_d = max([i for i, t in enumerate(d_tiles) if t["j"] != 1], default=-1)
    last_u2_g = max([i for i, t in enumerate(g_tiles) if t["j"] != 1], default=-1)
    dmul_per_rep = 1 + n_d

    # stripe/region mapping for psum slots
    def slot_addr(slot):
        return 32 * (slot % 3), (slot // 3) * 512

    nregs = [len([t for t in range(n_tiles) if t % 3 == s]) for s in range(3)]

    nc = bass.Bass("TRN2", target_bir_lowering=False, debug=False, num_devices=B)
    x_d = nc.dram_tensor("x", [C, HW], f32, kind="ExternalInput")
    w_d = nc.dram_tensor("w", [128, 24], f32, kind="ExternalInput")
    outa_d = nc.dram_tensor("out_a", [128, n_act], f32, kind="ExternalOutput")
    outp_d = nc.dram_tensor("out_p", [24, 1024], f32, kind="ExternalOutput")

    with contextlib.ExitStack() as st:
        Xf = st.enter_context(nc.sbuf_tensor("Xf", [128, FREE], f32))
        Us = [st.enter_context(nc.sbuf_tensor(f"U{i}", [128, FREE], bf16))
              for i in range(2)]
        U2s = [st.enter_context(nc.sbuf_tensor(f"U2{i}", [128, FREE], bf16))
               for i in range(2)]
        Anc = [st.enter_context(nc.sbuf_tensor(f"Anc{i}", [128, FREE], bf16))
               for i in range(NA)]
        Scr = [st.enter_context(nc.sbuf_tensor(f"Scr{i}", [128, FREE], bf16))
               for i in range(2)]
        Wd = [st.enter_context(nc.sbuf_tensor(f"Wd{i}", [128, FREE], bf16))
              for i in range(ND)]
        Wg = [st.enter_context(nc.sbuf_tensor(f"Wg{i}", [128, FREE], bf16))
              for i in range(NG)]
        wt = st.enter_context(nc.sbuf_tensor("wt", [128, 24], f32))
        onesb = st.enter_context(nc.sbuf_tensor("onesb", [128, 8], bf16))
        acta = st.enter_context(nc.sbuf_tensor("acta", [128, n_act], f32))
        Rs = st.enter_context(nc.sbuf_tensor("Rs", [128, 1024], f32))
        ps = st.enter_context(nc.psum_tensor("ps", [128, 4096], f32))

        s_dx0 = st.enter_context(nc.semaphore("s_dx0"))
        s_dx1 = st.enter_context(nc.semaphore("s_dx1"))
        s_dmw = st.enter_context(nc.semaphore("s_dmw"))
        s_u = st.enter_context(nc.semaphore("s_u"))
        s_anc = st.enter_context(nc.semaphore("s_anc"))
        s_md = st.enter_context(nc.semaphore("s_md"))
        s_mg = st.enter_context(nc.semaphore("s_mg"))
        s_pd = st.enter_context(nc.semaphore("s_pd"))
        s_pg = st.enter_context(nc.semaphore("s_pg"))
        s_ones = st.enter_context(nc.semaphore("s_ones"))
        s_out = st.enter_context(nc.semaphore("s_out"))
        s_cp = st.enter_context(nc.semaphore("s_cp"))

        block = st.enter_context(nc.Block())
        xr = x_d.ap().rearrange("c (g j) -> (c g) j", g=G)

        # ---------------- SP: x half 0 + final output DMAs ---------------
        @block.sync
        def _(sync):
            if not nodma:
                sync.dma_start(
                    Xf.ap()[:, 0 : FREE // 2], xr[:, 0 : FREE // 2]
                ).then_inc(s_dx0, 16)
            sync.wait_ge(s_cp, 3)
            sync.wait_ge(s_anc, reps * n_act)
            sync.dma_start(outa_d.ap(), acta.ap()).then_inc(s_out, 16)
            for stripe in range(3):
                nr = nregs[stripe]
                if nr == 0:
                    continue
                sync.dma_start(
                    outp_d.ap()[stripe * 8 : (stripe + 1) * 8, : nr * 128],
                    Rs.ap()[32 * stripe : 32 * stripe + 8, : nr * 128],
                ).then_inc(s_out, 16)

        # ---------------- GPSIMD: wt DMA + chain mults --------------------
        @block.gpsimd
        def _(gp):
            gp.dma_start(wt.ap(), w_d.ap()).then_inc(s_dmw, 16)
            for r in range(reps):
                seen_seg = set()
                for i, t in enumerate(g_tiles):
                    gi = r * n_g + i
                    if i == 0:
                        gp.wait_ge(s_u, r * 2 + 2)
                        if any(q["j"] != 1 for q in g_tiles):
                            # U2 of this rep ready (DVE op #0)
                            gp.wait_ge(s_md, r * dmul_per_rep + 1)
                    if t["seg"] not in seen_seg and t["j"] <= 2:
                        seen_seg.add(t["seg"])
                        gp.wait_ge(s_anc, r * n_act + t["seg"] + 1)
                    elif FULLSEMS and t["j"] <= 2:
                        gp.wait_ge(s_anc, r * n_act + t["seg"] + 1)
                    if FULLSEMS and t["j"] > 2:
                        prod = next(
                            k for k, q in enumerate(g_tiles)
                            if q["seg"] == t["seg"] and q["j"] == t["j"] - 2
                        )
                        gp.wait_ge(s_mg, r * n_g + prod + 1)
                    # ring reuse, batched
                    if gi >= NG and (
                        FULLSEMS or (gi - NG) % RING_BATCH_G == 0
                    ):
                        cover = min(
                            gi - NG + (1 if FULLSEMS else RING_BATCH_G) - 1,
                            reps * n_g - 1,
                        )
                        old = next(
                            q for q in pe_order
                            if q["eng"] == "G" and q["local"] == cover % n_g
                        )
                        gp.wait_ge(
                            s_pg, (cover // n_g) * n_g + old["pe_cum"]
                        )
                    if t["j"] <= 2:
                        src = Anc[(r * n_seg + t["seg"]) % NA].ap()
                    else:
                        prod = next(
                            k for k, q in enumerate(g_tiles)
                            if q["seg"] == t["seg"] and q["j"] == t["j"] - 2
                        )
                        src = Wg[(r * n_g + prod) % NG].ap()
                    mul = Us[r % 2].ap() if t["j"] == 1 else U2s[r % 2].ap()
                    nc.gpsimd.tensor_tensor(
                        Wg[gi % NG].ap(), src, mul, op=alu.mult
                    ).then_inc(s_mg, 1)

        # ---------------- ACT: x half 1 + u + anchors + directs ----------
        @block.scalar
        def _(scalar):
            if not nodma:
                scalar.dma_start(
                    Xf.ap()[:, FREE // 2 :], xr[:, FREE // 2 :]
                ).then_inc(s_dx1, 16)
                scalar.wait_ge(s_dx0, 16)
                scalar.wait_ge(s_dx1, 16)
            scalar.wait_ge(s_dmw, 16)
            for r in range(reps):
                for h in range(2):
                    if r > 1 and h == 0:
                        if last_u_d >= 0:
                            scalar.wait_ge(
                                s_md, (r - 2) * dmul_per_rep + 1 + last_u_d + 1
                            )
                        if last_u_g >= 0:
                            scalar.wait_ge(s_mg, (r - 2) * n_g + last_u_g + 1)
                    sl = slice(h * (FREE // 2), (h + 1) * (FREE // 2))
                    nc.scalar.activation(
                        Us[r % 2].ap()[:, sl], Xf.ap()[:, sl], act_fn.Exp,
                        scale=float(su), bias=wt.ap()[:, 9:10],
                    ).then_inc(s_u, 1)
                for si, (k0, chain) in enumerate(SEGMENTS):
                    pa = r * n_seg + si
                    if pa >= NA:
                        osi = (pa - NA) % n_seg
                        orr = (pa - NA) // n_seg
                        for (j, eng) in SEGMENTS[osi][1]:
                            if j > 2:
                                continue
                            if eng == "D":
                                li = next(
                                    k for k, q in enumerate(d_tiles)
                                    if q["seg"] == osi and q["j"] == j
                                )
                                scalar.wait_ge(
                                    s_md, orr * dmul_per_rep + 1 + li + 1
                                )
                            else:
                                li = next(
                                    k for k, q in enumerate(g_tiles)
                                    if q["seg"] == osi and q["j"] == j
                                )
                                scalar.wait_ge(s_mg, orr * n_g + li + 1)
                    nc.scalar.activation(
                        Anc[pa % NA].ap(), Xf.ap(), act_fn.Derivative_Erf,
                        scale=SQRT_A,
                        bias=wt.ap()[:, si : si + 1],
                        accum_out=acta.ap()[:, si : si + 1],
                    ).then_inc(s_anc, 1)
                for di, k in enumerate(DIRECT):
                    col = n_seg + di
                    if FULLSEMS and (r > 0 or di >= 2):
                        prev = r * n_act + n_seg + di - 2
                        if di < 2:
                            prev = (r - 1) * n_act + n_seg + di + 2
                        scalar.wait_ge(s_anc, prev + 1)
                    nc.scalar.activation(
                        Scr[di % 2].ap(), Xf.ap(), act_fn.Derivative_Erf,
                        scale=SQRT_A,
                        bias=wt.ap()[:, col : col + 1],
                        accum_out=acta.ap()[:, col : col + 1],
                    ).then_inc(s_anc, 1)

        # ---------------- DVE: ones copy + U2 + chain mults --------------
        @block.vector
        def _(vector):
            vector.wait_ge(s_dmw, 16)
            nc.vector.tensor_copy(onesb.ap(), wt.ap()[:, 16:24]).then_inc(
                s_ones, 1
            )
            for r in range(reps):
                vector.wait_ge(s_u, r * 2 + 2)
                if r > 1 and last_u2_g >= 0:
                    vector.wait_ge(s_mg, (r - 2) * n_g + last_u2_g + 1)
                if FULLSEMS and r > 1 and last_u2_d >= 0:
                    vector.wait_ge(
                        s_md, (r - 2) * dmul_per_rep + 1 + last_u2_d + 1
                    )
                nc.vector.tensor_tensor(
                    U2s[r % 2].ap(), Us[r % 2].ap(), Us[r % 2].ap(),
                    op=alu.mult,
                ).then_inc(s_md, 1)
                seen_seg = set()
                for i, t in enumerate(d_tiles):
                    gi = r * n_d + i
                    if t["seg"] not in seen_seg and t["j"] <= 2:
                        seen_seg.add(t["seg"])
                        vector.wait_ge(s_anc, r * n_act + t["seg"] + 1)
                    elif FULLSEMS and t["j"] <= 2:
                        vector.wait_ge(s_anc, r * n_act + t["seg"] + 1)
                    if FULLSEMS and t["j"] >= 2:
                        vector.wait_ge(s_md, r * dmul_per_rep + 1)
                    if FULLSEMS and t["j"] > 2:
                        prod = next(
                            k for k, q in enumerate(d_tiles)
                            if q["seg"] == t["seg"] and q["j"] == t["j"] - 2
                        )
                        vector.wait_ge(s_md, r * dmul_per_rep + 1 + prod + 1)
                    if gi >= ND and (
                        FULLSEMS or (gi - ND) % RING_BATCH_D == 0
                    ):
                        cover = min(
                            gi - ND + (1 if FULLSEMS else RING_BATCH_D) - 1,
                            reps * n_d - 1,
                        )
                        old = next(
                            q for q in pe_order
                            if q["eng"] == "D" and q["local"] == cover % n_d
                        )
                        vector.wait_ge(
                            s_pd, (cover // n_d) * n_d + old["pe_cum"]
                        )
                    if t["j"] <= 2:
                        src = Anc[(r * n_seg + t["seg"]) % NA].ap()
                    else:
                        prod = next(
                            k for k, q in enumerate(d_tiles)
                            if q["seg"] == t["seg"] and q["j"] == t["j"] - 2
                        )
                        src = Wd[(r * n_d + prod) % ND].ap()
                    mul = Us[r % 2].ap() if t["j"] == 1 else U2s[r % 2].ap()
                    nc.vector.tensor_tensor(
                        Wd[gi % ND].ap(), src, mul, op=alu.mult
                    ).then_inc(s_md, 1)
            # final: compact psum residual slots to SBUF for the out DMA
            vector.wait_ge(s_pd, reps * n_d)
            if n_g:
                vector.wait_ge(s_pg, reps * n_g)
            for stripe in range(3):
                nr = nregs[stripe]
                if nr == 0:
                    nc.vector.memset(
                        Rs.ap()[32 * stripe : 32 * stripe + 8, 0:1],
                        0.0,
                    ).then_inc(s_cp, 1)
                    continue
                psrc = ps.ap()[32 * stripe : 32 * stripe + 8, :].rearrange(
                    "p (r w) -> p r w", w=512
                )[:, :nr, 0:128]
                nc.vector.tensor_copy(
                    Rs.ap()[32 * stripe : 32 * stripe + 8, : nr * 128]
                    .rearrange("p (r w) -> p r w", w=128),
                    psrc,
                ).then_inc(s_cp, 1)

        # ---------------- PE: block-ones reduction into PSUM slots -------
        @block.tensor
        def _(tensor):
            tensor.wait_ge(s_ones, 1)
            for r in range(reps):
                for pi, t in enumerate(pe_order):
                    slot = t["slot"]
                    # producer progress, batched over PE_BATCH tiles
                    if FULLSEMS or pi % PE_BATCH == 0:
                        grp = pe_order[
                            pi : pi + (1 if FULLSEMS else PE_BATCH)
                        ]
                        need_d = max(
                            [q["local"] for q in grp if q["eng"] == "D"],
                            default=-1,
                        )
                        need_g = max(
                            [q["local"] for q in grp if q["eng"] == "G"],
                            default=-1,
                        )
                        if need_d >= 0:
                            tensor.wait_ge(
                                s_md, r * dmul_per_rep + 1 + need_d + 1
                            )
                        if need_g >= 0:
                            tensor.wait_ge(s_mg, r * n_g + need_g + 1)
                    if FULLSEMS and r > 0:
                        # cross-rep psum WAW self-edge (auto-true on HW)
                        if t["eng"] == "D":
                            tensor.wait_ge(
                                s_pd, (r - 1) * n_d + t["pe_cum"]
                            )
                        else:
                            tensor.wait_ge(
                                s_pg, (r - 1) * n_g + t["pe_cum"]
                            )
                    bp, fo = slot_addr(slot)
                    if t["eng"] == "D":
                        w = Wd[(r * n_d + t["local"]) % ND].ap()
                    else:
                        w = Wg[(r * n_g + t["local"]) % NG].ap()
                    for q in range(32):
                        mm = nc.tensor.matmul(
                            ps.ap()[bp : bp + 8, fo : fo + 128],
                            onesb.ap(),
                            w[:, q * 128 : (q + 1) * 128],
                            start=(q == 0), stop=(q == 31),
                        )
                    if t["eng"] == "D":
                        mm.then_inc(s_pd, 1)
                    else:
                        mm.then_inc(s_pg, 1)

    _nc_cache[key] = nc
    return nc


def _build_w(bin_centers=None) -> np.ndarray:
    if bin_centers is None:
        bin_centers = np.linspace(0.0, 1.0, NBINS)
    bc = np.asarray(bin_centers, np.float64)
    w = np.zeros((128, 24), np.float32)
    act_bins = [k0 for k0, _ in SEGMENTS] + list(DIRECT)
    for i, k in enumerate(act_bins):
        w[:, i] = np.float32(-SQRT_A * bc[k])
    delta = (bc[-1] - bc[0]) / (NBINS - 1)
    w[:, 9] = np.float32(-2.0 * A_COEF * delta * GAMMA)
    for c in range(C):
        w[c * G : (c + 1) * G, 16 + c] = 1.0
    return w


def _host_combine(acta: np.ndarray, outp: np.ndarray, bc: np.ndarray) -> np.ndarray:
    """acta [128, n_act]; outp [24, 1024] (psum residuals) -> [C, NBINS]."""
    tiles, _d, _g, pe_order, _nd, _ng = _plan()
    drift, _su, _bu, _delta = _drift(bc)
    out = np.zeros((C, NBINS), np.float64)
    scale = (ER / RATIO) * (math.sqrt(math.pi) / 2.0)
    act_bins = [k0 for k0, _ in SEGMENTS] + list(DIRECT)
    a = acta.reshape(C, G, -1).sum(axis=1)
    for i, k in enumerate(act_bins):
        out[:, k] = a[:, i] * scale
    for t in pe_order:
        k = t["bin"]
        s = t["slot"]
        stripe, region = s % 3, s // 3
        vals = outp[stripe * 8 : stripe * 8 + C,
                    region * 128 : (region + 1) * 128].sum(axis=1)
        out[:, k] = vals * scale * math.exp(-drift[k])
    return out.astype(np.float32)


def kernel(x: np.ndarray, bin_centers: np.ndarray) -> np.ndarray:
    global last_results
    x = np.ascontiguousarray(np.asarray(x), dtype=np.float32)
    bc = np.asarray(bin_centers, np.float64)
    assert x.shape == (B, C, 256, 256), x.shape
    assert bc.shape == (NBINS,), bc.shape

    nc = _build(bc)
    w = _build_w(bc)
    in_maps = [{"x": x[b].reshape(C, HW), "w": w} for b in range(B)]
    res = run_bass_kernel_spmd(nc, in_maps, list(range(B)))
    last_results = res
    outs = []
    for b in range(B):
        acta = np.asarray(res.results[b]["out_a"], np.float64)
        outp = np.asarray(res.results[b]["out_p"], np.float64)
        outs.append(_host_combine(acta, outp, bc))
    return np.stack(outs).reshape(B, C * NBINS, 1, 1).astype(np.float32)


# revision 20
# speedup vs baseline: 1.0215x; 1.0215x over previous
"""Trainium2 Bass kernel for nn_DiffHistogram (Gaussian soft-binned histogram).

out[b, c*32+k, 0, 0] = sum_{h,w} (ER/RATIO) * exp(-(x-c_k)^2 / (2*sigma^2))
x: [8, 8, 256, 256] f32, bin centers equally spaced on [0, 1].

Sharding: data-parallel over batch B across 8 NeuronCores; per-core SBUF
layout [128, 4096], partition p = c*16+g.

Algorithm (anchor + geometric chain, multi-engine):
  Equal bin spacing D gives w_{k+1}(x) = w_k(x)*u(x) with
  u = exp(2*a*D*(x-gamma)) up to a known per-bin constant (host-folded).
  - ACT: u via Exp (2 half passes), anchor/direct bins via Derivative_Erf
    (accum_out = that bin's per-partition sums for free).
  - DVE (+ optionally GPSIMD): one bf16 tensor_tensor multiply per chain
    bin (~1.8us at 2x mode). Chains split into odd/even subchains via
    U2 = u^2.
  - PE: block-ones matmul reduces each chain tile into a PSUM slot
    (32 chunk matmuls, ~1.7us/bin); the [8, 128] residuals are DMA'd to
    HBM at the end and summed on the host.

HW-CRITICAL: on this hardware a DVE/GP/PE `wait_ge` costs ~1.6us EVEN IF
ALREADY SATISFIED (measured: bf16 TT 1813ns -> 6567ns with +3 waits).
So the HW build ("lean" mode) emits only the cross-engine waits required
for correctness, batched where possible. CoreSim's race detector needs
edges for every hazard incl. same-engine, so sim builds (DIFFHIST_FULLSEMS
=1, set by test.py) add the always-true self-edges back.

Numerics: bf16 chains + f32 accumulation, rel err ~1.7e-4 (tol 2e-2).
"""

import contextlib
import math
import os

import numpy as np

import concourse.bass as bass
import concourse.mybir as mybir
from concourse.bass_utils import run_bass_kernel_spmd

B = 8
C = 8
HW = 256 * 256
NBINS = 32
G = 128 // C
FREE = HW // G          # 4096

ER = 1.0
RATIO = 2.5066
SIGMA = 1.0 / NBINS
A_COEF = 1.0 / (2.0 * SIGMA * SIGMA)       # 512.0
SQRT_A = math.sqrt(A_COEF)                 # 22.627417
GAMMA = 0.5

FULLSEMS = os.environ.get("DIFFHIST_FULLSEMS", "0") == "1"

_CFG = os.environ.get("DIFFHIST_CFG", "v1")
if _CFG == "v1":
    SEGMENTS = [
        (12, [(1, "D"), (2, "G"), (3, "D"), (4, "G"), (5, "D"), (6, "G"), (7, "D")]),
        (4,  [(1, "D"), (2, "G"), (3, "D"), (4, "G"), (5, "D"), (6, "G"), (7, "D")]),
        (22, [(1, "D"), (2, "G"), (3, "D"), (4, "G")]),
        (0,  [(1, "D"), (2, "G"), (3, "D")]),
        (29, [(1, "D"), (2, "D")]),
    ]
    DIRECT = [20, 21, 27, 28]
elif _CFG == "nogp":
    SEGMENTS = [
        (12, [(1, "D"), (2, "D"), (3, "D"), (4, "D"), (5, "D"), (6, "D"), (7, "D")]),
        (4,  [(1, "D"), (2, "D"), (3, "D"), (4, "D"), (5, "D"), (6, "D"), (7, "D")]),
        (22, [(1, "D"), (2, "D"), (3, "D"), (4, "D")]),
        (0,  [(1, "D"), (2, "D"), (3, "D")]),
        (29, [(1, "D"), (2, "D")]),
    ]
    DIRECT = [20, 21, 27, 28]
elif _CFG == "gp6":
    SEGMENTS = [
        (12, [(1, "D"), (2, "G"), (3, "D"), (4, "G"), (5, "D"), (6, "G"), (7, "D")]),
        (4,  [(1, "D"), (2, "G"), (3, "D"), (4, "G"), (5, "D"), (6, "G"), (7, "D")]),
        (22, [(1, "D"), (2, "D"), (3, "D"), (4, "D")]),
        (0,  [(1, "D"), (2, "D"), (3, "D")]),
        (29, [(1, "D"), (2, "D")]),
    ]
    DIRECT = [20, 21, 27, 28]
else:
    raise ValueError(_CFG)

ND = 6
NG = 4
NA = 4
RING_BATCH_D = 3
RING_BATCH_G = 2
PE_BATCH = 2

COST_D = 1900.0
COST_G = 3500.0
COST_PE = 1750.0
COST_ACT = 3900.0

_nc_cache: dict = {}
last_results = None


def _plan():
    tiles = []
    nd = ng = 0
    for si, (k0, chain) in enumerate(SEGMENTS):
        for (j, eng) in chain:
            t = {"bin": k0 + j, "seg": si, "j": j, "eng": eng}
            if eng == "D":
                t["local"] = nd
                nd += 1
            else:
                t["local"] = ng
                ng += 1
            tiles.append(t)
    n_d, n_g = nd, ng
    d_tiles = [t for t in tiles if t["eng"] == "D"]
    g_tiles = [t for t in tiles if t["eng"] == "G"]

    # estimated completion times, for PE consumption order only
    t_u = 7400.0 + 2 * 1200.0
    anchor_done = {}
    tact = t_u + 1300.0
    for si in range(len(SEGMENTS)):
        tact += COST_ACT
        anchor_done[si] = tact
    tile_done = {}
    tdve = t_u + COST_D
    for t in d_tiles:
        dep = anchor_done[t["seg"]] if t["j"] <= 2 else tile_done[(t["seg"], t["j"] - 2)]
        tdve = max(tdve, dep) + COST_D
        tile_done[(t["seg"], t["j"])] = tdve
        t["est"] = tdve
    tgp = t_u
    for t in g_tiles:
        dep = anchor_done[t["seg"]] if t["j"] <= 2 else tile_done[(t["seg"], t["j"] - 2)]
        tgp = max(tgp, dep) + COST_G
        tile_done[(t["seg"], t["j"])] = tgp
        t["est"] = tgp

    pe_order = sorted(tiles, key=lambda t: t["est"])
    cd = cg = 0
    for slot, t in enumerate(pe_order):
        t["slot"] = slot
        if t["eng"] == "D":
            cd += 1
            t["pe_cum"] = cd
        else:
            cg += 1
            t["pe_cum"] = cg
    return tiles, d_tiles, g_tiles, pe_order, n_d, n_g


def _drift(bc: np.ndarray):
    bc = np.asarray(bc, np.float64)
    delta = (bc[-1] - bc[0]) / (NBINS - 1)
    su = 2.0 * A_COEF * delta
    bu = -su * GAMMA
    out = {}
    for k0, chain in SEGMENTS:
        c0 = bc[k0]
        for (j, _e) in chain:
            out[k0 + j] = A_COEF * ((c0 + j * delta) ** 2 - c0 ** 2) + j * bu
    return out, su, bu, delta


def _build(bin_centers: np.ndarray, reps: int = 1) -> "bass.Bass":
    bc = np.asarray(bin_centers, np.float64)
    nodma = os.environ.get("DIFFHIST_NODMA", "0") == "1"
    key = (reps, nodma, FULLSEMS, _CFG, tuple(bc.tolist()))
    if key in _nc_cache:
        return _nc_cache[key]

    tiles, d_tiles, g_tiles, pe_order, n_d, n_g = _plan()
    n_tiles = len(tiles)
    n_seg = len(SEGMENTS)
    n_act = n_seg + len(DIRECT)
    _dr, su, bu, delta = _drift(bc)

    f32 = mybir.dt.float32
    bf16 = mybir.dt.bfloat16
    alu = mybir.AluOpType
    act_fn = mybir.ActivationFunctionType

    last_u_d = max([i for i, t in enumerate(d_tiles) if t["j"] == 1], default=-1)
    last_u_g = max([i for i, t in enumerate(g_tiles) if t["j"] == 1], default=-1)
    last_u2_d = max([i for i, t in enumerate(d_tiles) if t["j"] != 1], default=-1)
    last_u2_g = max([i for i, t in enumerate(g_tiles) if t["j"] != 1], default=-1)
    dmul_per_rep = 1 + n_d

    # stripe/region mapping for psum slots
    def slot_addr(slot):
        return 32 * (slot % 3), (slot // 3) * 512

    nregs = [len([t for t in range(n_tiles) if t % 3 == s]) for s in range(3)]

    nc = bass.Bass("TRN2", target_bir_lowering=False, debug=False, num_devices=B)
    x_d = nc.dram_tensor("x", [C, HW], f32, kind="ExternalInput")
    w_d = nc.dram_tensor("w", [128, 24], f32, kind="ExternalInput")
    outa_d = nc.dram_tensor("out_a", [128, n_act], f32, kind="ExternalOutput")
    outp_d = nc.dram_tensor("out_p", [24, 4096], f32, kind="ExternalOutput")

    with contextlib.ExitStack() as st:
        Xf = st.enter_context(nc.sbuf_tensor("Xf", [128, FREE], f32))
        Us = [st.enter_context(nc.sbuf_tensor(f"U{i}", [128, FREE], bf16))
              for i in range(2)]
        U2s = [st.enter_context(nc.sbuf_tensor(f"U2{i}", [128, FREE], bf16))
               for i in range(2)]
        Anc = [st.enter_context(nc.sbuf_tensor(f"Anc{i}", [128, FREE], bf16))
               for i in range(NA)]
        Scr = [st.enter_context(nc.sbuf_tensor(f"Scr{i}", [128, FREE], bf16))
               for i in range(2)]
        Wd = [st.enter_context(nc.sbuf_tensor(f"Wd{i}", [128, FREE], bf16))
              for i in range(ND)]
        Wg = [st.enter_context(nc.sbuf_tensor(f"Wg{i}", [128, FREE], bf16))
              for i in range(NG)]
        wt = st.enter_context(nc.sbuf_tensor("wt", [128, 24], f32))
        onesb = st.enter_context(nc.sbuf_tensor("onesb", [128, 8], bf16))
        acta = st.enter_context(nc.sbuf_tensor("acta", [128, n_act], f32))
        Rs = st.enter_context(nc.sbuf_tensor("Rs", [128, 4096], f32))
        ps = st.enter_context(nc.psum_tensor("ps", [128, 4096], f32))

        s_dx0 = st.enter_context(nc.semaphore("s_dx0"))
        s_dx1 = st.enter_context(nc.semaphore("s_dx1"))
        s_dmw = st.enter_context(nc.semaphore("s_dmw"))
        s_u = st.enter_context(nc.semaphore("s_u"))
        s_anc = st.enter_context(nc.semaphore("s_anc"))
        s_md = st.enter_context(nc.semaphore("s_md"))
        s_mg = st.enter_context(nc.semaphore("s_mg"))
        s_pd = st.enter_context(nc.semaphore("s_pd"))
        s_pg = st.enter_context(nc.semaphore("s_pg"))
        s_ones = st.enter_context(nc.semaphore("s_ones"))
        s_out = st.enter_context(nc.semaphore("s_out"))
        s_cp = st.enter_context(nc.semaphore("s_cp"))

        block = st.enter_context(nc.Block())
        xr = x_d.ap().rearrange("c (g j) -> (c g) j", g=G)

        # ---------------- SP: x half 0 + final output DMAs ---------------
        @block.sync
        def _(sync):
            if not nodma:
                sync.dma_start(
                    Xf.ap()[:, 0 : FREE // 2], xr[:, 0 : FREE // 2]
                ).then_inc(s_dx0, 16)
            sync.wait_ge(s_cp, 3)
            sync.wait_ge(s_anc, reps * n_act)
            sync.dma_start(outa_d.ap(), acta.ap()).then_inc(s_out, 16)
            for stripe in range(3):
                nr = nregs[stripe]
                if nr == 0:
                    continue
                sync.dma_start(
                    outp_d.ap()[stripe * 8 : (stripe + 1) * 8, : nr * 512],
                    Rs.ap()[32 * stripe : 32 * stripe + 8, : nr * 512],
                ).then_inc(s_out, 16)

        # ---------------- GPSIMD: wt DMA + chain mults --------------------
        @block.gpsimd
        def _(gp):
            gp.dma_start(wt.ap(), w_d.ap()).then_inc(s_dmw, 16)
            for r in range(reps):
                seen_seg = set()
                for i, t in enumerate(g_tiles):
                    gi = r * n_g + i
                    if i == 0:
                        gp.wait_ge(s_u, r * 2 + 2)
                        if any(q["j"] != 1 for q in g_tiles):
                            # U2 of this rep ready (DVE op #0)
                            gp.wait_ge(s_md, r * dmul_per_rep + 1)
                    if t["seg"] not in seen_seg and t["j"] <= 2:
                        seen_seg.add(t["seg"])
                        gp.wait_ge(s_anc, r * n_act + t["seg"] + 1)
                    elif FULLSEMS and t["j"] <= 2:
                        gp.wait_ge(s_anc, r * n_act + t["seg"] + 1)
                    if FULLSEMS and t["j"] > 2:
                        prod = next(
                            k for k, q in enumerate(g_tiles)
                            if q["seg"] == t["seg"] and q["j"] == t["j"] - 2
                        )
                        gp.wait_ge(s_mg, r * n_g + prod + 1)
                    # ring reuse, batched
                    if gi >= NG and (
                        FULLSEMS or (gi - NG) % RING_BATCH_G == 0
                    ):
                        cover = min(
                            gi - NG + (1 if FULLSEMS else RING_BATCH_G) - 1,
                            reps * n_g - 1,
                        )
                        old = next(
                            q for q in pe_order
                            if q["eng"] == "G" and q["local"] == cover % n_g
                        )
                        gp.wait_ge(
                            s_pg, (cover // n_g) * n_g + old["pe_cum"]
                        )
                    if t["j"] <= 2:
                        src = Anc[(r * n_seg + t["seg"]) % NA].ap()
                    else:
                        prod = next(
                            k for k, q in enumerate(g_tiles)
                            if q["seg"] == t["seg"] and q["j"] == t["j"] - 2
                        )
                        src = Wg[(r * n_g + prod) % NG].ap()
                    mul = Us[r % 2].ap() if t["j"] == 1 else U2s[r % 2].ap()
                    nc.gpsimd.tensor_tensor(
                        Wg[gi % NG].ap(), src, mul, op=alu.mult
                    ).then_inc(s_mg, 1)

        # ---------------- ACT: x half 1 + u + anchors + directs ----------
        @block.scalar
        def _(scalar):
            if not nodma:
                scalar.dma_start(
                    Xf.ap()[:, FREE // 2 :], xr[:, FREE // 2 :]
                ).then_inc(s_dx1, 16)
                scalar.wait_ge(s_dx0, 16)
                scalar.wait_ge(s_dx1, 16)
            scalar.wait_ge(s_dmw, 16)
            for r in range(reps):
                for h in range(2):
                    if r > 1 and h == 0:
                        if last_u_d >= 0:
                            scalar.wait_ge(
                                s_md, (r - 2) * dmul_per_rep + 1 + last_u_d + 1
                            )
                        if last_u_g >= 0:
                            scalar.wait_ge(s_mg, (r - 2) * n_g + last_u_g + 1)
                    sl = slice(h * (FREE // 2), (h + 1) * (FREE // 2))
                    nc.scalar.activation(
                        Us[r % 2].ap()[:, sl], Xf.ap()[:, sl], act_fn.Exp,
                        scale=float(su), bias=wt.ap()[:, 9:10],
                    ).then_inc(s_u, 1)
                for si, (k0, chain) in enumerate(SEGMENTS):
                    pa = r * n_seg + si
                    if pa >= NA:
                        osi = (pa - NA) % n_seg
                        orr = (pa - NA) // n_seg
                        for (j, eng) in SEGMENTS[osi][1]:
                            if j > 2:
                                continue
                            if eng == "D":
                                li = next(
                                    k for k, q in enumerate(d_tiles)
                                    if q["seg"] == osi and q["j"] == j
                                )
                                scalar.wait_ge(
                                    s_md, orr * dmul_per_rep + 1 + li + 1
                                )
                            else:
                                li = next(
                                    k for k, q in enumerate(g_tiles)
                                    if q["seg"] == osi and q["j"] == j
                                )
                                scalar.wait_ge(s_mg, orr * n_g + li + 1)
                    nc.scalar.activation(
                        Anc[pa % NA].ap(), Xf.ap(), act_fn.Derivative_Erf,
                        scale=SQRT_A,
                        bias=wt.ap()[:, si : si + 1],
                        accum_out=acta.ap()[:, si : si + 1],
                    ).then_inc(s_anc, 1)
                for di, k in enumerate(DIRECT):
                    col = n_seg + di
                    if FULLSEMS and (r > 0 or di >= 2):
                        prev = r * n_act + n_seg + di - 2
                        if di < 2:
                            prev = (r - 1) * n_act + n_seg + di + 2
                        scalar.wait_ge(s_anc, prev + 1)
                    nc.scalar.activation(
                        Scr[di % 2].ap(), Xf.ap(), act_fn.Derivative_Erf,
                        scale=SQRT_A,
                        bias=wt.ap()[:, col : col + 1],
                        accum_out=acta.ap()[:, col : col + 1],
                    ).then_inc(s_anc, 1)

        # ---------------- DVE: ones copy + U2 + chain mults --------------
        @block.vector
        def _(vector):
            vector.wait_ge(s_dmw, 16)
            nc.vector.tensor_copy(onesb.ap(), wt.ap()[:, 16:24]).then_inc(
                s_ones, 1
            )
            for r in range(reps):
                vector.wait_ge(s_u, r * 2 + 2)
                if r > 1 and last_u2_g >= 0:
                    vector.wait_ge(s_mg, (r - 2) * n_g + last_u2_g + 1)
                if FULLSEMS and r > 1 and last_u2_d >= 0:
                    vector.wait_ge(
                        s_md, (r - 2) * dmul_per_rep + 1 + last_u2_d + 1
                    )
                nc.vector.tensor_tensor(
                    U2s[r % 2].ap(), Us[r % 2].ap(), Us[r % 2].ap(),
                    op=alu.mult,
                ).then_inc(s_md, 1)
                seen_seg = set()
                for i, t in enumerate(d_tiles):
                    gi = r * n_d + i
                    if t["seg"] not in seen_seg and t["j"] <= 2:
                        seen_seg.add(t["seg"])
                        vector.wait_ge(s_anc, r * n_act + t["seg"] + 1)
                    elif FULLSEMS and t["j"] <= 2:
                        vector.wait_ge(s_anc, r * n_act + t["seg"] + 1)
                    if FULLSEMS and t["j"] >= 2:
                        vector.wait_ge(s_md, r * dmul_per_rep + 1)
                    if FULLSEMS and t["j"] > 2:
                        prod = next(
                            k for k, q in enumerate(d_tiles)
                            if q["seg"] == t["seg"] and q["j"] == t["j"] - 2
                        )
                        vector.wait_ge(s_md, r * dmul_per_rep + 1 + prod + 1)
                    if gi >= ND and (
                        FULLSEMS or (gi - ND) % RING_BATCH_D == 0
                    ):
                        cover = min(
                            gi - ND + (1 if FULLSEMS else RING_BATCH_D) - 1,
                            reps * n_d - 1,
                        )
                        old = next(
                            q for q in pe_order
                            if q["eng"] == "D" and q["local"] == cover % n_d
                        )
                        vector.wait_ge(
                            s_pd, (cover // n_d) * n_d + old["pe_cum"]
                        )
                    if t["j"] <= 2:
                        src = Anc[(r * n_seg + t["seg"]) % NA].ap()
                    else:
                        prod = next(
                            k for k, q in enumerate(d_tiles)
                            if q["seg"] == t["seg"] and q["j"] == t["j"] - 2
                        )
                        src = Wd[(r * n_d + prod) % ND].ap()
                    mul = Us[r % 2].ap() if t["j"] == 1 else U2s[r % 2].ap()
                    nc.vector.tensor_tensor(
                        Wd[gi % ND].ap(), src, mul, op=alu.mult
                    ).then_inc(s_md, 1)
            # final: compact psum residual slots to SBUF for the out DMA
            vector.wait_ge(s_pd, reps * n_d)
            if n_g:
                vector.wait_ge(s_pg, reps * n_g)
            for stripe in range(3):
                nr = nregs[stripe]
                if nr == 0:
                    nc.vector.memset(
                        Rs.ap()[32 * stripe : 32 * stripe + 8, 0:1],
                        0.0,
                    ).then_inc(s_cp, 1)
                    continue
                nc.vector.tensor_copy(
                    Rs.ap()[32 * stripe : 32 * stripe + 8, : nr * 512],
                    ps.ap()[32 * stripe : 32 * stripe + 8, : nr * 512],
                ).then_inc(s_cp, 1)

        # ---------------- PE: block-ones reduction into PSUM slots -------
        @block.tensor
        def _(tensor):
            tensor.wait_ge(s_ones, 1)
            for r in range(reps):
                for pi, t in enumerate(pe_order):
                    slot = t["slot"]
                    # producer progress, batched over PE_BATCH tiles
                    if FULLSEMS or pi % PE_BATCH == 0:
                        grp = pe_order[
                            pi : pi + (1 if FULLSEMS else PE_BATCH)
                        ]
                        need_d = max(
                            [q["local"] for q in grp if q["eng"] == "D"],
                            default=-1,
                        )
                        need_g = max(
                            [q["local"] for q in grp if q["eng"] == "G"],
                            default=-1,
                        )
                        if need_d >= 0:
                            tensor.wait_ge(
                                s_md, r * dmul_per_rep + 1 + need_d + 1
                            )
                        if need_g >= 0:
                            tensor.wait_ge(s_mg, r * n_g + need_g + 1)
                    if FULLSEMS and r > 0:
                        # cross-rep psum WAW self-edge (auto-true on HW)
                        if t["eng"] == "D":
                            tensor.wait_ge(
                                s_pd, (r - 1) * n_d + t["pe_cum"]
                            )
                        else:
                            tensor.wait_ge(
                                s_pg, (r - 1) * n_g + t["pe_cum"]
                            )
                    bp, fo = slot_addr(slot)
                    if t["eng"] == "D":
                        w = Wd[(r * n_d + t["local"]) % ND].ap()
                    else:
                        w = Wg[(r * n_g + t["local"]) % NG].ap()
                    for q in range(8):
                        mm = nc.tensor.matmul(
                            ps.ap()[bp : bp + 8, fo : fo + 512],
                            onesb.ap(),
                            w[:, q * 512 : (q + 1) * 512],
                            start=(q == 0), stop=(q == 7),
                        )
                    if t["eng"] == "D":
                        mm.then_inc(s_pd, 1)
                    else:
                        mm.then_inc(s_pg, 1)

    _nc_cache[key] = nc
    return nc


def _build_w(bin_centers=None) -> np.ndarray:
    if bin_centers is None:
        bin_centers = np.linspace(0.0, 1.0, NBINS)
    bc = np.asarray(bin_centers, np.float64)
    w = np.zeros((128, 24), np.float32)
    act_bins = [k0 for k0, _ in SEGMENTS] + list(DIRECT)
    for i, k in enumerate(act_bins):
        w[:, i] = np.float32(-SQRT_A * bc[k])
    delta = (bc[-1] - bc[0]) / (NBINS - 1)
    w[:, 9] = np.float32(-2.0 * A_COEF * delta * GAMMA)
    for c in range(C):
        w[c * G : (c + 1) * G, 16 + c] = 1.0
    return w


def _host_combine(acta: np.ndarray, outp: np.ndarray, bc: np.ndarray) -> np.ndarray:
    """acta [128, n_act]; outp [24, 1024] (psum residuals) -> [C, NBINS]."""
    tiles, _d, _g, pe_order, _nd, _ng = _plan()
    drift, _su, _bu, _delta = _drift(bc)
    out = np.zeros((C, NBINS), np.float64)
    scale = (ER / RATIO) * (math.sqrt(math.pi) / 2.0)
    act_bins = [k0 for k0, _ in SEGMENTS] + list(DIRECT)
    a = acta.reshape(C, G, -1).sum(axis=1)
    for i, k in enumerate(act_bins):
        out[:, k] = a[:, i] * scale
    for t in pe_order:
        k = t["bin"]
        s = t["slot"]
        stripe, region = s % 3, s // 3
        vals = outp[stripe * 8 : stripe * 8 + C,
                    region * 512 : (region + 1) * 512].sum(axis=1)
        out[:, k] = vals * scale * math.exp(-drift[k])
    return out.astype(np.float32)


def kernel(x: np.ndarray, bin_centers: np.ndarray) -> np.ndarray:
    global last_results
    x = np.ascontiguousarray(np.asarray(x), dtype=np.float32)
    bc = np.asarray(bin_centers, np.float64)
    assert x.shape == (B, C, 256, 256), x.shape
    assert bc.shape == (NBINS,), bc.shape

    nc = _build(bc)
    w = _build_w(bc)
    in_maps = [{"x": x[b].reshape(C, HW), "w": w} for b in range(B)]
    res = run_bass_kernel_spmd(nc, in_maps, list(range(B)))
    last_results = res
    outs = []
    for b in range(B):
        acta = np.asarray(res.results[b]["out_a"], np.float64)
        outp = np.asarray(res.results[b]["out_p"], np.float64)
        outs.append(_host_combine(acta, outp, bc))
    return np.stack(outs).reshape(B, C * NBINS, 1, 1).astype(np.float32)


# revision 21
# speedup vs baseline: 1.8987x; 1.8588x over previous
"""Trainium2 Bass kernel for nn_DiffHistogram (Gaussian soft-binned histogram).

out[b, c*32+k, 0, 0] = sum_{h,w} (ER/RATIO) * exp(-(x-c_k)^2 / (2*sigma^2))
x: [8, 8, 256, 256] f32, bin centers equally spaced on [0, 1].

Sharding: data-parallel over batch B across 8 NeuronCores; per-core SBUF
layout [128, 4096], partition p = c*16+g.

Algorithm (anchor + geometric chain, multi-engine):
  Equal bin spacing D gives w_{k+1}(x) = w_k(x)*u(x) with
  u = exp(2*a*D*(x-gamma)) up to a known per-bin constant (host-folded).
  - ACT: u via Exp (2 half passes), anchor/direct bins via Derivative_Erf
    (accum_out = that bin's per-partition sums for free).
  - DVE (+ optionally GPSIMD): one bf16 tensor_tensor multiply per chain
    bin (~1.8us at 2x mode). Chains split into odd/even subchains via
    U2 = u^2.
  - PE: block-ones matmul reduces each chain tile into a PSUM slot
    (32 chunk matmuls, ~1.7us/bin); the [8, 128] residuals are DMA'd to
    HBM at the end and summed on the host.

HW-CRITICAL: on this hardware a DVE/GP/PE `wait_ge` costs ~1.6us EVEN IF
ALREADY SATISFIED (measured: bf16 TT 1813ns -> 6567ns with +3 waits).
So the HW build ("lean" mode) emits only the cross-engine waits required
for correctness, batched where possible. CoreSim's race detector needs
edges for every hazard incl. same-engine, so sim builds (DIFFHIST_FULLSEMS
=1, set by test.py) add the always-true self-edges back.

Numerics: bf16 chains + f32 accumulation, rel err ~1.7e-4 (tol 2e-2).
"""

import contextlib
import math
import os

import numpy as np

import concourse.bass as bass
import concourse.mybir as mybir
from concourse.bass_utils import run_bass_kernel_spmd

B = 8
C = 8
HW = 256 * 256
NBINS = 32
G = 128 // C
FREE = HW // G          # 4096

ER = 1.0
RATIO = 2.5066
SIGMA = 1.0 / NBINS
A_COEF = 1.0 / (2.0 * SIGMA * SIGMA)       # 512.0
SQRT_A = math.sqrt(A_COEF)                 # 22.627417
GAMMA = 0.5

FULLSEMS = os.environ.get("DIFFHIST_FULLSEMS", "0") == "1"

_CFG = os.environ.get("DIFFHIST_CFG", "nogp")
if _CFG == "v1":
    SEGMENTS = [
        (12, [(1, "D"), (2, "G"), (3, "D"), (4, "G"), (5, "D"), (6, "G"), (7, "D")]),
        (4,  [(1, "D"), (2, "G"), (3, "D"), (4, "G"), (5, "D"), (6, "G"), (7, "D")]),
        (22, [(1, "D"), (2, "G"), (3, "D"), (4, "G")]),
        (0,  [(1, "D"), (2, "G"), (3, "D")]),
        (29, [(1, "D"), (2, "D")]),
    ]
    DIRECT = [20, 21, 27, 28]
elif _CFG == "nogp":
    SEGMENTS = [
        (12, [(1, "D"), (2, "D"), (3, "D"), (4, "D"), (5, "D"), (6, "D"), (7, "D")]),
        (4,  [(1, "D"), (2, "D"), (3, "D"), (4, "D"), (5, "D"), (6, "D"), (7, "D")]),
        (22, [(1, "D"), (2, "D"), (3, "D"), (4, "D")]),
        (0,  [(1, "D"), (2, "D"), (3, "D")]),
        (29, [(1, "D"), (2, "D")]),
    ]
    DIRECT = [20, 21, 27, 28]
elif _CFG == "gp6":
    SEGMENTS = [
        (12, [(1, "D"), (2, "G"), (3, "D"), (4, "G"), (5, "D"), (6, "G"), (7, "D")]),
        (4,  [(1, "D"), (2, "G"), (3, "D"), (4, "G"), (5, "D"), (6, "G"), (7, "D")]),
        (22, [(1, "D"), (2, "D"), (3, "D"), (4, "D")]),
        (0,  [(1, "D"), (2, "D"), (3, "D")]),
        (29, [(1, "D"), (2, "D")]),
    ]
    DIRECT = [20, 21, 27, 28]
else:
    raise ValueError(_CFG)

ND = 6
NG = 4
NA = 4
RING_BATCH_D = 3
RING_BATCH_G = 2
PE_BATCH = 2

COST_D = 1900.0
COST_G = 3500.0
COST_PE = 1750.0
COST_ACT = 3900.0

_nc_cache: dict = {}
last_results = None


def _plan():
    tiles = []
    nd = ng = 0
    for si, (k0, chain) in enumerate(SEGMENTS):
        for (j, eng) in chain:
            t = {"bin": k0 + j, "seg": si, "j": j, "eng": eng}
            if eng == "D":
                t["local"] = nd
                nd += 1
            else:
                t["local"] = ng
                ng += 1
            tiles.append(t)
    n_d, n_g = nd, ng
    d_tiles = [t for t in tiles if t["eng"] == "D"]
    g_tiles = [t for t in tiles if t["eng"] == "G"]

    # estimated completion times, for PE consumption order only
    t_u = 7400.0 + 2 * 1200.0
    anchor_done = {}
    tact = t_u + 1300.0
    for si in range(len(SEGMENTS)):
        tact += COST_ACT
        anchor_done[si] = tact
    tile_done = {}
    tdve = t_u + COST_D
    for t in d_tiles:
        dep = anchor_done[t["seg"]] if t["j"] <= 2 else tile_done[(t["seg"], t["j"] - 2)]
        tdve = max(tdve, dep) + COST_D
        tile_done[(t["seg"], t["j"])] = tdve
        t["est"] = tdve
    tgp = t_u
    for t in g_tiles:
        dep = anchor_done[t["seg"]] if t["j"] <= 2 else tile_done[(t["seg"], t["j"] - 2)]
        tgp = max(tgp, dep) + COST_G
        tile_done[(t["seg"], t["j"])] = tgp
        t["est"] = tgp

    pe_order = sorted(tiles, key=lambda t: t["est"])
    cd = cg = 0
    for slot, t in enumerate(pe_order):
        t["slot"] = slot
        if t["eng"] == "D":
            cd += 1
            t["pe_cum"] = cd
        else:
            cg += 1
            t["pe_cum"] = cg
    return tiles, d_tiles, g_tiles, pe_order, n_d, n_g


def _drift(bc: np.ndarray):
    bc = np.asarray(bc, np.float64)
    delta = (bc[-1] - bc[0]) / (NBINS - 1)
    su = 2.0 * A_COEF * delta
    bu = -su * GAMMA
    out = {}
    for k0, chain in SEGMENTS:
        c0 = bc[k0]
        for (j, _e) in chain:
            out[k0 + j] = A_COEF * ((c0 + j * delta) ** 2 - c0 ** 2) + j * bu
    return out, su, bu, delta


def _build(bin_centers: np.ndarray, reps: int = 1) -> "bass.Bass":
    bc = np.asarray(bin_centers, np.float64)
    nodma = os.environ.get("DIFFHIST_NODMA", "0") == "1"
    key = (reps, nodma, FULLSEMS, _CFG, tuple(bc.tolist()))
    if key in _nc_cache:
        return _nc_cache[key]

    tiles, d_tiles, g_tiles, pe_order, n_d, n_g = _plan()
    n_tiles = len(tiles)
    n_seg = len(SEGMENTS)
    n_act = n_seg + len(DIRECT)
    _dr, su, bu, delta = _drift(bc)

    f32 = mybir.dt.float32
    bf16 = mybir.dt.bfloat16
    alu = mybir.AluOpType
    act_fn = mybir.ActivationFunctionType

    last_u_d = max([i for i, t in enumerate(d_tiles) if t["j"] == 1], default=-1)
    last_u_g = max([i for i, t in enumerate(g_tiles) if t["j"] == 1], default=-1)
    last_u2_d = max([i for i, t in enumerate(d_tiles) if t["j"] != 1], default=-1)
    last_u2_g = max([i for i, t in enumerate(g_tiles) if t["j"] != 1], default=-1)
    dmul_per_rep = 1 + n_d

    # stripe/region mapping for psum slots
    def slot_addr(slot):
        return 32 * (slot % 3), (slot // 3) * 512

    nregs = [len([t for t in range(n_tiles) if t % 3 == s]) for s in range(3)]

    nc = bass.Bass("TRN2", target_bir_lowering=False, debug=False, num_devices=B)
    x_d = nc.dram_tensor("x", [C, HW], f32, kind="ExternalInput")
    w_d = nc.dram_tensor("w", [128, 24], f32, kind="ExternalInput")
    outa_d = nc.dram_tensor("out_a", [128, n_act], f32, kind="ExternalOutput")
    outp_d = nc.dram_tensor("out_p", [24, 4096], f32, kind="ExternalOutput")

    with contextlib.ExitStack() as st:
        Xf = st.enter_context(nc.sbuf_tensor("Xf", [128, FREE], f32))
        Us = [st.enter_context(nc.sbuf_tensor(f"U{i}", [128, FREE], bf16))
              for i in range(2)]
        U2s = [st.enter_context(nc.sbuf_tensor(f"U2{i}", [128, FREE], bf16))
               for i in range(2)]
        Anc = [st.enter_context(nc.sbuf_tensor(f"Anc{i}", [128, FREE], bf16))
               for i in range(NA)]
        Scr = [st.enter_context(nc.sbuf_tensor(f"Scr{i}", [128, FREE], bf16))
               for i in range(2)]
        Wd = [st.enter_context(nc.sbuf_tensor(f"Wd{i}", [128, FREE], bf16))
              for i in range(ND)]
        Wg = [st.enter_context(nc.sbuf_tensor(f"Wg{i}", [128, FREE], bf16))
              for i in range(NG)]
        wt = st.enter_context(nc.sbuf_tensor("wt", [128, 24], f32))
        onesb = st.enter_context(nc.sbuf_tensor("onesb", [128, 8], bf16))
        acta = st.enter_context(nc.sbuf_tensor("acta", [128, n_act], f32))
        Rs = st.enter_context(nc.sbuf_tensor("Rs", [128, 4096], f32))
        ps = st.enter_context(nc.psum_tensor("ps", [128, 4096], f32))

        s_dx0 = st.enter_context(nc.semaphore("s_dx0"))
        s_dx1 = st.enter_context(nc.semaphore("s_dx1"))
        s_dmw = st.enter_context(nc.semaphore("s_dmw"))
        s_u = st.enter_context(nc.semaphore("s_u"))
        s_anc = st.enter_context(nc.semaphore("s_anc"))
        s_md = st.enter_context(nc.semaphore("s_md"))
        s_mg = st.enter_context(nc.semaphore("s_mg"))
        s_pd = st.enter_context(nc.semaphore("s_pd"))
        s_pg = st.enter_context(nc.semaphore("s_pg"))
        s_ones = st.enter_context(nc.semaphore("s_ones"))
        s_out = st.enter_context(nc.semaphore("s_out"))
        s_cp = st.enter_context(nc.semaphore("s_cp"))

        block = st.enter_context(nc.Block())
        xr = x_d.ap().rearrange("c (g j) -> (c g) j", g=G)

        # ---------------- SP: x half 0 + final output DMAs ---------------
        @block.sync
        def _(sync):
            if not nodma:
                sync.dma_start(
                    Xf.ap()[:, 0 : FREE // 2], xr[:, 0 : FREE // 2]
                ).then_inc(s_dx0, 16)
            sync.wait_ge(s_cp, 3)
            sync.wait_ge(s_anc, reps * n_act)
            sync.dma_start(outa_d.ap(), acta.ap()).then_inc(s_out, 16)
            for stripe in range(3):
                nr = nregs[stripe]
                if nr == 0:
                    continue
                sync.dma_start(
                    outp_d.ap()[stripe * 8 : (stripe + 1) * 8, : nr * 512],
                    Rs.ap()[32 * stripe : 32 * stripe + 8, : nr * 512],
                ).then_inc(s_out, 16)

        # ---------------- GPSIMD: wt DMA + chain mults --------------------
        @block.gpsimd
        def _(gp):
            gp.dma_start(wt.ap(), w_d.ap()).then_inc(s_dmw, 16)
            for r in range(reps):
                seen_seg = set()
                for i, t in enumerate(g_tiles):
                    gi = r * n_g + i
                    if i == 0:
                        gp.wait_ge(s_u, r * 2 + 2)
                        if any(q["j"] != 1 for q in g_tiles):
                            # U2 of this rep ready (DVE op #0)
                            gp.wait_ge(s_md, r * dmul_per_rep + 1)
                    if t["seg"] not in seen_seg and t["j"] <= 2:
                        seen_seg.add(t["seg"])
                        gp.wait_ge(s_anc, r * n_act + t["seg"] + 1)
                    elif FULLSEMS and t["j"] <= 2:
                        gp.wait_ge(s_anc, r * n_act + t["seg"] + 1)
                    if FULLSEMS and t["j"] > 2:
                        prod = next(
                            k for k, q in enumerate(g_tiles)
                            if q["seg"] == t["seg"] and q["j"] == t["j"] - 2
                        )
                        gp.wait_ge(s_mg, r * n_g + prod + 1)
                    # ring reuse, batched
                    if gi >= NG and (
                        FULLSEMS or (gi - NG) % RING_BATCH_G == 0
                    ):
                        cover = min(
                            gi - NG + (1 if FULLSEMS else RING_BATCH_G) - 1,
                            reps * n_g - 1,
                        )
                        old = next(
                            q for q in pe_order
                            if q["eng"] == "G" and q["local"] == cover % n_g
                        )
                        gp.wait_ge(
                            s_pg, (cover // n_g) * n_g + old["pe_cum"]
                        )
                    if t["j"] <= 2:
                        src = Anc[(r * n_seg + t["seg"]) % NA].ap()
                    else:
                        prod = next(
                            k for k, q in enumerate(g_tiles)
                            if q["seg"] == t["seg"] and q["j"] == t["j"] - 2
                        )
                        src = Wg[(r * n_g + prod) % NG].ap()
                    mul = Us[r % 2].ap() if t["j"] == 1 else U2s[r % 2].ap()
                    nc.gpsimd.tensor_tensor(
                        Wg[gi % NG].ap(), src, mul, op=alu.mult
                    ).then_inc(s_mg, 1)

        # ---------------- ACT: x half 1 + u + anchors + directs ----------
        @block.scalar
        def _(scalar):
            if not nodma:
                scalar.dma_start(
                    Xf.ap()[:, FREE // 2 :], xr[:, FREE // 2 :]
                ).then_inc(s_dx1, 16)
                scalar.wait_ge(s_dx0, 16)
                scalar.wait_ge(s_dx1, 16)
            scalar.wait_ge(s_dmw, 16)
            for r in range(reps):
                for h in range(2):
                    if r > 1 and h == 0:
                        if last_u_d >= 0:
                            scalar.wait_ge(
                                s_md, (r - 2) * dmul_per_rep + 1 + last_u_d + 1
                            )
                        if last_u_g >= 0:
                            scalar.wait_ge(s_mg, (r - 2) * n_g + last_u_g + 1)
                    sl = slice(h * (FREE // 2), (h + 1) * (FREE // 2))
                    nc.scalar.activation(
                        Us[r % 2].ap()[:, sl], Xf.ap()[:, sl], act_fn.Exp,
                        scale=float(su), bias=wt.ap()[:, 9:10],
                    ).then_inc(s_u, 1)
                for si, (k0, chain) in enumerate(SEGMENTS):
                    pa = r * n_seg + si
                    if pa >= NA:
                        osi = (pa - NA) % n_seg
                        orr = (pa - NA) // n_seg
                        for (j, eng) in SEGMENTS[osi][1]:
                            if j > 2:
                                continue
                            if eng == "D":
                                li = next(
                                    k for k, q in enumerate(d_tiles)
                                    if q["seg"] == osi and q["j"] == j
                                )
                                scalar.wait_ge(
                                    s_md, orr * dmul_per_rep + 1 + li + 1
                                )
                            else:
                                li = next(
                                    k for k, q in enumerate(g_tiles)
                                    if q["seg"] == osi and q["j"] == j
                                )
                                scalar.wait_ge(s_mg, orr * n_g + li + 1)
                    nc.scalar.activation(
                        Anc[pa % NA].ap(), Xf.ap(), act_fn.Derivative_Erf,
                        scale=SQRT_A,
                        bias=wt.ap()[:, si : si + 1],
                        accum_out=acta.ap()[:, si : si + 1],
                    ).then_inc(s_anc, 1)
                for di, k in enumerate(DIRECT):
                    col = n_seg + di
                    if FULLSEMS and (r > 0 or di >= 2):
                        prev = r * n_act + n_seg + di - 2
                        if di < 2:
                            prev = (r - 1) * n_act + n_seg + di + 2
                        scalar.wait_ge(s_anc, prev + 1)
                    nc.scalar.activation(
                        Scr[di % 2].ap(), Xf.ap(), act_fn.Derivative_Erf,
                        scale=SQRT_A,
                        bias=wt.ap()[:, col : col + 1],
                        accum_out=acta.ap()[:, col : col + 1],
                    ).then_inc(s_anc, 1)

        # ---------------- DVE: ones copy + U2 + chain mults --------------
        @block.vector
        def _(vector):
            vector.wait_ge(s_dmw, 16)
            nc.vector.tensor_copy(onesb.ap(), wt.ap()[:, 16:24]).then_inc(
                s_ones, 1
            )
            for r in range(reps):
                vector.wait_ge(s_u, r * 2 + 2)
                if r > 1 and last_u2_g >= 0:
                    vector.wait_ge(s_mg, (r - 2) * n_g + last_u2_g + 1)
                if FULLSEMS and r > 1 and last_u2_d >= 0:
                    vector.wait_ge(
                        s_md, (r - 2) * dmul_per_rep + 1 + last_u2_d + 1
                    )
                nc.vector.tensor_tensor(
                    U2s[r % 2].ap(), Us[r % 2].ap(), Us[r % 2].ap(),
                    op=alu.mult,
                ).then_inc(s_md, 1)
                seen_seg = set()
                for i, t in enumerate(d_tiles):
                    gi = r * n_d + i
                    if t["seg"] not in seen_seg and t["j"] <= 2:
                        seen_seg.add(t["seg"])
                        vector.wait_ge(s_anc, r * n_act + t["seg"] + 1)
                    elif FULLSEMS and t["j"] <= 2:
                        vector.wait_ge(s_anc, r * n_act + t["seg"] + 1)
                    if FULLSEMS and t["j"] >= 2:
                        vector.wait_ge(s_md, r * dmul_per_rep + 1)
                    if FULLSEMS and t["j"] > 2:
                        prod = next(
                            k for k, q in enumerate(d_tiles)
                            if q["seg"] == t["seg"] and q["j"] == t["j"] - 2
                        )
                        vector.wait_ge(s_md, r * dmul_per_rep + 1 + prod + 1)
                    if gi >= ND and (
                        FULLSEMS or (gi - ND) % RING_BATCH_D == 0
                    ):
                        cover = min(
                            gi - ND + (1 if FULLSEMS else RING_BATCH_D) - 1,
                            reps * n_d - 1,
                        )
                        old = next(
                            q for q in pe_order
                            if q["eng"] == "D" and q["local"] == cover % n_d
                        )
                        vector.wait_ge(
                            s_pd, (cover // n_d) * n_d + old["pe_cum"]
                        )
                    if t["j"] <= 2:
                        src = Anc[(r * n_seg + t["seg"]) % NA].ap()
                    else:
                        prod = next(
                            k for k, q in enumerate(d_tiles)
                            if q["seg"] == t["seg"] and q["j"] == t["j"] - 2
                        )
                        src = Wd[(r * n_d + prod) % ND].ap()
                    mul = Us[r % 2].ap() if t["j"] == 1 else U2s[r % 2].ap()
                    nc.vector.tensor_tensor(
                        Wd[gi % ND].ap(), src, mul, op=alu.mult
                    ).then_inc(s_md, 1)
            # final: compact psum residual slots to SBUF for the out DMA
            vector.wait_ge(s_pd, reps * n_d)
            if n_g:
                vector.wait_ge(s_pg, reps * n_g)
            for stripe in range(3):
                nr = nregs[stripe]
                if nr == 0:
                    nc.vector.memset(
                        Rs.ap()[32 * stripe : 32 * stripe + 8, 0:1],
                        0.0,
                    ).then_inc(s_cp, 1)
                    continue
                nc.vector.tensor_copy(
                    Rs.ap()[32 * stripe : 32 * stripe + 8, : nr * 512],
                    ps.ap()[32 * stripe : 32 * stripe + 8, : nr * 512],
                ).then_inc(s_cp, 1)

        # ---------------- PE: block-ones reduction into PSUM slots -------
        @block.tensor
        def _(tensor):
            tensor.wait_ge(s_ones, 1)
            for r in range(reps):
                for pi, t in enumerate(pe_order):
                    slot = t["slot"]
                    # producer progress, batched over PE_BATCH tiles
                    if FULLSEMS or pi % PE_BATCH == 0:
                        grp = pe_order[
                            pi : pi + (1 if FULLSEMS else PE_BATCH)
                        ]
                        need_d = max(
                            [q["local"] for q in grp if q["eng"] == "D"],
                            default=-1,
                        )
                        need_g = max(
                            [q["local"] for q in grp if q["eng"] == "G"],
                            default=-1,
                        )
                        if need_d >= 0:
                            tensor.wait_ge(
                                s_md, r * dmul_per_rep + 1 + need_d + 1
                            )
                        if need_g >= 0:
                            tensor.wait_ge(s_mg, r * n_g + need_g + 1)
                    if FULLSEMS and r > 0:
                        # cross-rep psum WAW self-edge (auto-true on HW)
                        if t["eng"] == "D":
                            tensor.wait_ge(
                                s_pd, (r - 1) * n_d + t["pe_cum"]
                            )
                        else:
                            tensor.wait_ge(
                                s_pg, (r - 1) * n_g + t["pe_cum"]
                            )
                    bp, fo = slot_addr(slot)
                    if t["eng"] == "D":
                        w = Wd[(r * n_d + t["local"]) % ND].ap()
                    else:
                        w = Wg[(r * n_g + t["local"]) % NG].ap()
                    for q in range(8):
                        mm = nc.tensor.matmul(
                            ps.ap()[bp : bp + 8, fo : fo + 512],
                            onesb.ap(),
                            w[:, q * 512 : (q + 1) * 512],
                            start=(q == 0), stop=(q == 7),
                        )
                    if t["eng"] == "D":
                        mm.then_inc(s_pd, 1)
                    else:
                        mm.then_inc(s_pg, 1)

    _nc_cache[key] = nc
    return nc


def _build_w(bin_centers=None) -> np.ndarray:
    if bin_centers is None:
        bin_centers = np.linspace(0.0, 1.0, NBINS)
    bc = np.asarray(bin_centers, np.float64)
    w = np.zeros((128, 24), np.float32)
    act_bins = [k0 for k0, _ in SEGMENTS] + list(DIRECT)
    for i, k in enumerate(act_bins):
        w[:, i] = np.float32(-SQRT_A * bc[k])
    delta = (bc[-1] - bc[0]) / (NBINS - 1)
    w[:, 9] = np.float32(-2.0 * A_COEF * delta * GAMMA)
    for c in range(C):
        w[c * G : (c + 1) * G, 16 + c] = 1.0
    return w


def _host_combine(acta: np.ndarray, outp: np.ndarray, bc: np.ndarray) -> np.ndarray:
    """acta [128, n_act]; outp [24, 1024] (psum residuals) -> [C, NBINS]."""
    tiles, _d, _g, pe_order, _nd, _ng = _plan()
    drift, _su, _bu, _delta = _drift(bc)
    out = np.zeros((C, NBINS), np.float64)
    scale = (ER / RATIO) * (math.sqrt(math.pi) / 2.0)
    act_bins = [k0 for k0, _ in SEGMENTS] + list(DIRECT)
    a = acta.reshape(C, G, -1).sum(axis=1)
    for i, k in enumerate(act_bins):
        out[:, k] = a[:, i] * scale
    for t in pe_order:
        k = t["bin"]
        s = t["slot"]
        stripe, region = s % 3, s // 3
        vals = outp[stripe * 8 : stripe * 8 + C,
                    region * 512 : (region + 1) * 512].sum(axis=1)
        out[:, k] = vals * scale * math.exp(-drift[k])
    return out.astype(np.float32)


def kernel(x: np.ndarray, bin_centers: np.ndarray) -> np.ndarray:
    global last_results
    x = np.ascontiguousarray(np.asarray(x), dtype=np.float32)
    bc = np.asarray(bin_centers, np.float64)
    assert x.shape == (B, C, 256, 256), x.shape
    assert bc.shape == (NBINS,), bc.shape

    nc = _build(bc)
    w = _build_w(bc)
    in_maps = [{"x": x[b].reshape(C, HW), "w": w} for b in range(B)]
    res = run_bass_kernel_spmd(nc, in_maps, list(range(B)))
    last_results = res
    outs = []
    for b in range(B):
        acta = np.asarray(res.results[b]["out_a"], np.float64)
        outp = np.asarray(res.results[b]["out_p"], np.float64)
        outs.append(_host_combine(acta, outp, bc))
    return np.stack(outs).reshape(B, C * NBINS, 1, 1).astype(np.float32)


# revision 22
# speedup vs baseline: 2.2476x; 1.1837x over previous
"""Trainium2 Bass kernel for nn_DiffHistogram (Gaussian soft-binned histogram).

out[b, c*32+k, 0, 0] = sum_{h,w} (ER/RATIO) * exp(-(x-c_k)^2 / (2*sigma^2))
x: [8, 8, 256, 256] f32, bin centers equally spaced on [0, 1].

Sharding: data-parallel over batch B across 8 NeuronCores; per-core SBUF
layout [128, 4096], partition p = c*16+g.

Algorithm (anchor + geometric chain, multi-engine):
  Equal bin spacing D gives w_{k+1}(x) = w_k(x)*u(x) with
  u = exp(2*a*D*(x-gamma)) up to a known per-bin constant (host-folded).
  - ACT: u via Exp (2 half passes), anchor/direct bins via Derivative_Erf
    (accum_out = that bin's per-partition sums for free).
  - DVE (+ optionally GPSIMD): one bf16 tensor_tensor multiply per chain
    bin (~1.8us at 2x mode). Chains split into odd/even subchains via
    U2 = u^2.
  - PE: block-ones matmul reduces each chain tile into a PSUM slot
    (32 chunk matmuls, ~1.7us/bin); the [8, 128] residuals are DMA'd to
    HBM at the end and summed on the host.

HW-CRITICAL: on this hardware a DVE/GP/PE `wait_ge` costs ~1.6us EVEN IF
ALREADY SATISFIED (measured: bf16 TT 1813ns -> 6567ns with +3 waits).
So the HW build ("lean" mode) emits only the cross-engine waits required
for correctness, batched where possible. CoreSim's race detector needs
edges for every hazard incl. same-engine, so sim builds (DIFFHIST_FULLSEMS
=1, set by test.py) add the always-true self-edges back.

Numerics: bf16 chains + f32 accumulation, rel err ~1.7e-4 (tol 2e-2).
"""

import contextlib
import math
import os

import numpy as np

import concourse.bass as bass
import concourse.mybir as mybir
from concourse.bass_utils import run_bass_kernel_spmd

B = 8
C = 8
HW = 256 * 256
NBINS = 32
G = 128 // C
FREE = HW // G          # 4096

ER = 1.0
RATIO = 2.5066
SIGMA = 1.0 / NBINS
A_COEF = 1.0 / (2.0 * SIGMA * SIGMA)       # 512.0
SQRT_A = math.sqrt(A_COEF)                 # 22.627417
GAMMA = 0.5

FULLSEMS = os.environ.get("DIFFHIST_FULLSEMS", "0") == "1"

_CFG = os.environ.get("DIFFHIST_CFG", "nogp")
if _CFG == "v1":
    SEGMENTS = [
        (12, [(1, "D"), (2, "G"), (3, "D"), (4, "G"), (5, "D"), (6, "G"), (7, "D")]),
        (4,  [(1, "D"), (2, "G"), (3, "D"), (4, "G"), (5, "D"), (6, "G"), (7, "D")]),
        (22, [(1, "D"), (2, "G"), (3, "D"), (4, "G")]),
        (0,  [(1, "D"), (2, "G"), (3, "D")]),
        (29, [(1, "D"), (2, "D")]),
    ]
    DIRECT = [20, 21, 27, 28]
elif _CFG == "nogp":
    SEGMENTS = [
        (12, [(1, "D"), (2, "D"), (3, "D"), (4, "D"), (5, "D"), (6, "D"), (7, "D")]),
        (4,  [(1, "D"), (2, "D"), (3, "D"), (4, "D"), (5, "D"), (6, "D"), (7, "D")]),
        (22, [(1, "D"), (2, "D"), (3, "D"), (4, "D")]),
        (0,  [(1, "D"), (2, "D"), (3, "D")]),
        (29, [(1, "D"), (2, "D")]),
    ]
    DIRECT = [20, 21, 27, 28]
elif _CFG == "gp6":
    SEGMENTS = [
        (12, [(1, "D"), (2, "G"), (3, "D"), (4, "G"), (5, "D"), (6, "G"), (7, "D")]),
        (4,  [(1, "D"), (2, "G"), (3, "D"), (4, "G"), (5, "D"), (6, "G"), (7, "D")]),
        (22, [(1, "D"), (2, "D"), (3, "D"), (4, "D")]),
        (0,  [(1, "D"), (2, "D"), (3, "D")]),
        (29, [(1, "D"), (2, "D")]),
    ]
    DIRECT = [20, 21, 27, 28]
else:
    raise ValueError(_CFG)

_N_G = sum(1 for _k, _ch in SEGMENTS for (_j, _e) in _ch if _e == "G")
if _N_G == 0:
    ND = 10          # Wg buffers freed -> bigger DVE ring, fewer waits
    RING_BATCH_D = 5
    PE_BATCH = 4
else:
    ND = 6
    RING_BATCH_D = 3
    PE_BATCH = 2
NG = 4
NA = 4
RING_BATCH_G = 2

COST_D = 1900.0
COST_G = 3500.0
COST_PE = 1750.0
COST_ACT = 3900.0

_nc_cache: dict = {}
last_results = None


def _plan():
    tiles = []
    nd = ng = 0
    for si, (k0, chain) in enumerate(SEGMENTS):
        for (j, eng) in chain:
            t = {"bin": k0 + j, "seg": si, "j": j, "eng": eng}
            if eng == "D":
                t["local"] = nd
                nd += 1
            else:
                t["local"] = ng
                ng += 1
            tiles.append(t)
    n_d, n_g = nd, ng
    d_tiles = [t for t in tiles if t["eng"] == "D"]
    g_tiles = [t for t in tiles if t["eng"] == "G"]

    # estimated completion times, for PE consumption order only
    t_u = 7400.0 + 2 * 1200.0
    anchor_done = {}
    tact = t_u + 1300.0
    for si in range(len(SEGMENTS)):
        tact += COST_ACT
        anchor_done[si] = tact
    tile_done = {}
    tdve = t_u + COST_D
    for t in d_tiles:
        dep = anchor_done[t["seg"]] if t["j"] <= 2 else tile_done[(t["seg"], t["j"] - 2)]
        tdve = max(tdve, dep) + COST_D
        tile_done[(t["seg"], t["j"])] = tdve
        t["est"] = tdve
    tgp = t_u
    for t in g_tiles:
        dep = anchor_done[t["seg"]] if t["j"] <= 2 else tile_done[(t["seg"], t["j"] - 2)]
        tgp = max(tgp, dep) + COST_G
        tile_done[(t["seg"], t["j"])] = tgp
        t["est"] = tgp

    pe_order = sorted(tiles, key=lambda t: t["est"])
    cd = cg = 0
    for slot, t in enumerate(pe_order):
        t["slot"] = slot
        if t["eng"] == "D":
            cd += 1
            t["pe_cum"] = cd
        else:
            cg += 1
            t["pe_cum"] = cg
    return tiles, d_tiles, g_tiles, pe_order, n_d, n_g


def _drift(bc: np.ndarray):
    bc = np.asarray(bc, np.float64)
    delta = (bc[-1] - bc[0]) / (NBINS - 1)
    su = 2.0 * A_COEF * delta
    bu = -su * GAMMA
    out = {}
    for k0, chain in SEGMENTS:
        c0 = bc[k0]
        for (j, _e) in chain:
            out[k0 + j] = A_COEF * ((c0 + j * delta) ** 2 - c0 ** 2) + j * bu
    return out, su, bu, delta


def _build(bin_centers: np.ndarray, reps: int = 1) -> "bass.Bass":
    bc = np.asarray(bin_centers, np.float64)
    nodma = os.environ.get("DIFFHIST_NODMA", "0") == "1"
    key = (reps, nodma, FULLSEMS, _CFG, tuple(bc.tolist()))
    if key in _nc_cache:
        return _nc_cache[key]

    tiles, d_tiles, g_tiles, pe_order, n_d, n_g = _plan()
    n_tiles = len(tiles)
    n_seg = len(SEGMENTS)
    n_act = n_seg + len(DIRECT)
    _dr, su, bu, delta = _drift(bc)

    f32 = mybir.dt.float32
    bf16 = mybir.dt.bfloat16
    alu = mybir.AluOpType
    act_fn = mybir.ActivationFunctionType

    last_u_d = max([i for i, t in enumerate(d_tiles) if t["j"] == 1], default=-1)
    last_u_g = max([i for i, t in enumerate(g_tiles) if t["j"] == 1], default=-1)
    last_u2_d = max([i for i, t in enumerate(d_tiles) if t["j"] != 1], default=-1)
    last_u2_g = max([i for i, t in enumerate(g_tiles) if t["j"] != 1], default=-1)
    dmul_per_rep = 1 + n_d

    # stripe/region mapping for psum slots
    def slot_addr(slot):
        return 32 * (slot % 3), (slot // 3) * 512

    nregs = [len([t for t in range(n_tiles) if t % 3 == s]) for s in range(3)]

    nc = bass.Bass("TRN2", target_bir_lowering=False, debug=False, num_devices=B)
    x_d = nc.dram_tensor("x", [C, HW], f32, kind="ExternalInput")
    w_d = nc.dram_tensor("w", [128, 24], f32, kind="ExternalInput")
    outa_d = nc.dram_tensor("out_a", [128, n_act], f32, kind="ExternalOutput")
    outp_d = nc.dram_tensor("out_p", [24, 4096], f32, kind="ExternalOutput")

    with contextlib.ExitStack() as st:
        Xf = st.enter_context(nc.sbuf_tensor("Xf", [128, FREE], f32))
        Us = [st.enter_context(nc.sbuf_tensor(f"U{i}", [128, FREE], bf16))
              for i in range(2)]
        U2s = [st.enter_context(nc.sbuf_tensor(f"U2{i}", [128, FREE], bf16))
               for i in range(2)]
        Anc = [st.enter_context(nc.sbuf_tensor(f"Anc{i}", [128, FREE], bf16))
               for i in range(NA)]
        Scr = [st.enter_context(nc.sbuf_tensor(f"Scr{i}", [128, FREE], bf16))
               for i in range(2)]
        Wd = [st.enter_context(nc.sbuf_tensor(f"Wd{i}", [128, FREE], bf16))
              for i in range(ND)]
        Wg = [st.enter_context(nc.sbuf_tensor(f"Wg{i}", [128, FREE], bf16))
              for i in range(NG if n_g else 0)]
        wt = st.enter_context(nc.sbuf_tensor("wt", [128, 24], f32))
        onesb = st.enter_context(nc.sbuf_tensor("onesb", [128, 8], bf16))
        acta = st.enter_context(nc.sbuf_tensor("acta", [128, n_act], f32))
        Rs = st.enter_context(nc.sbuf_tensor("Rs", [128, 4096], f32))
        ps = st.enter_context(nc.psum_tensor("ps", [128, 4096], f32))

        s_dx0 = st.enter_context(nc.semaphore("s_dx0"))
        s_dx1 = st.enter_context(nc.semaphore("s_dx1"))
        s_dmw = st.enter_context(nc.semaphore("s_dmw"))
        s_u = st.enter_context(nc.semaphore("s_u"))
        s_anc = st.enter_context(nc.semaphore("s_anc"))
        s_md = st.enter_context(nc.semaphore("s_md"))
        s_mg = st.enter_context(nc.semaphore("s_mg"))
        s_pd = st.enter_context(nc.semaphore("s_pd"))
        s_pg = st.enter_context(nc.semaphore("s_pg"))
        s_ones = st.enter_context(nc.semaphore("s_ones"))
        s_out = st.enter_context(nc.semaphore("s_out"))
        s_cp = st.enter_context(nc.semaphore("s_cp"))

        block = st.enter_context(nc.Block())
        xr = x_d.ap().rearrange("c (g j) -> (c g) j", g=G)

        # ---------------- SP: x half 0 + final output DMAs ---------------
        @block.sync
        def _(sync):
            if not nodma:
                sync.dma_start(
                    Xf.ap()[:, 0 : FREE // 2], xr[:, 0 : FREE // 2]
                ).then_inc(s_dx0, 16)
            sync.wait_ge(s_cp, 3)
            sync.wait_ge(s_anc, reps * n_act)
            sync.dma_start(outa_d.ap(), acta.ap()).then_inc(s_out, 16)
            for stripe in range(3):
                nr = nregs[stripe]
                if nr == 0:
                    continue
                sync.dma_start(
                    outp_d.ap()[stripe * 8 : (stripe + 1) * 8, : nr * 512],
                    Rs.ap()[32 * stripe : 32 * stripe + 8, : nr * 512],
                ).then_inc(s_out, 16)

        # ---------------- GPSIMD: wt DMA + chain mults --------------------
        @block.gpsimd
        def _(gp):
            gp.dma_start(wt.ap(), w_d.ap()).then_inc(s_dmw, 16)
            for r in range(reps):
                seen_seg = set()
                for i, t in enumerate(g_tiles):
                    gi = r * n_g + i
                    if i == 0:
                        gp.wait_ge(s_u, r * 2 + 2)
                        if any(q["j"] != 1 for q in g_tiles):
                            # U2 of this rep ready (DVE op #0)
                            gp.wait_ge(s_md, r * dmul_per_rep + 1)
                    if t["seg"] not in seen_seg and t["j"] <= 2:
                        seen_seg.add(t["seg"])
                        gp.wait_ge(s_anc, r * n_act + t["seg"] + 1)
                    elif FULLSEMS and t["j"] <= 2:
                        gp.wait_ge(s_anc, r * n_act + t["seg"] + 1)
                    if FULLSEMS and t["j"] > 2:
                        prod = next(
                            k for k, q in enumerate(g_tiles)
                            if q["seg"] == t["seg"] and q["j"] == t["j"] - 2
                        )
                        gp.wait_ge(s_mg, r * n_g + prod + 1)
                    # ring reuse, batched
                    if gi >= NG and (
                        FULLSEMS or (gi - NG) % RING_BATCH_G == 0
                    ):
                        cover = min(
                            gi - NG + (1 if FULLSEMS else RING_BATCH_G) - 1,
                            reps * n_g - 1,
                        )
                        old = next(
                            q for q in pe_order
                            if q["eng"] == "G" and q["local"] == cover % n_g
                        )
                        gp.wait_ge(
                            s_pg, (cover // n_g) * n_g + old["pe_cum"]
                        )
                    if t["j"] <= 2:
                        src = Anc[(r * n_seg + t["seg"]) % NA].ap()
                    else:
                        prod = next(
                            k for k, q in enumerate(g_tiles)
                            if q["seg"] == t["seg"] and q["j"] == t["j"] - 2
                        )
                        src = Wg[(r * n_g + prod) % NG].ap()
                    mul = Us[r % 2].ap() if t["j"] == 1 else U2s[r % 2].ap()
                    nc.gpsimd.tensor_tensor(
                        Wg[gi % NG].ap(), src, mul, op=alu.mult
                    ).then_inc(s_mg, 1)

        # ---------------- ACT: x half 1 + u + anchors + directs ----------
        @block.scalar
        def _(scalar):
            if not nodma:
                scalar.dma_start(
                    Xf.ap()[:, FREE // 2 :], xr[:, FREE // 2 :]
                ).then_inc(s_dx1, 16)
                scalar.wait_ge(s_dx0, 16)
                scalar.wait_ge(s_dx1, 16)
            scalar.wait_ge(s_dmw, 16)
            for r in range(reps):
                for h in range(2):
                    if r > 1 and h == 0:
                        if last_u_d >= 0:
                            scalar.wait_ge(
                                s_md, (r - 2) * dmul_per_rep + 1 + last_u_d + 1
                            )
                        if last_u_g >= 0:
                            scalar.wait_ge(s_mg, (r - 2) * n_g + last_u_g + 1)
                    sl = slice(h * (FREE // 2), (h + 1) * (FREE // 2))
                    nc.scalar.activation(
                        Us[r % 2].ap()[:, sl], Xf.ap()[:, sl], act_fn.Exp,
                        scale=float(su), bias=wt.ap()[:, 9:10],
                    ).then_inc(s_u, 1)
                for si, (k0, chain) in enumerate(SEGMENTS):
                    pa = r * n_seg + si
                    if pa >= NA:
                        osi = (pa - NA) % n_seg
                        orr = (pa - NA) // n_seg
                        for (j, eng) in SEGMENTS[osi][1]:
                            if j > 2:
                                continue
                            if eng == "D":
                                li = next(
                                    k for k, q in enumerate(d_tiles)
                                    if q["seg"] == osi and q["j"] == j
                                )
                                scalar.wait_ge(
                                    s_md, orr * dmul_per_rep + 1 + li + 1
                                )
                            else:
                                li = next(
                                    k for k, q in enumerate(g_tiles)
                                    if q["seg"] == osi and q["j"] == j
                                )
                                scalar.wait_ge(s_mg, orr * n_g + li + 1)
                    nc.scalar.activation(
                        Anc[pa % NA].ap(), Xf.ap(), act_fn.Derivative_Erf,
                        scale=SQRT_A,
                        bias=wt.ap()[:, si : si + 1],
                        accum_out=acta.ap()[:, si : si + 1],
                    ).then_inc(s_anc, 1)
                for di, k in enumerate(DIRECT):
                    col = n_seg + di
                    if FULLSEMS and (r > 0 or di >= 2):
                        prev = r * n_act + n_seg + di - 2
                        if di < 2:
                            prev = (r - 1) * n_act + n_seg + di + 2
                        scalar.wait_ge(s_anc, prev + 1)
                    nc.scalar.activation(
                        Scr[di % 2].ap(), Xf.ap(), act_fn.Derivative_Erf,
                        scale=SQRT_A,
                        bias=wt.ap()[:, col : col + 1],
                        accum_out=acta.ap()[:, col : col + 1],
                    ).then_inc(s_anc, 1)

        # ---------------- DVE: ones copy + U2 + chain mults --------------
        @block.vector
        def _(vector):
            vector.wait_ge(s_dmw, 16)
            nc.vector.tensor_copy(onesb.ap(), wt.ap()[:, 16:24]).then_inc(
                s_ones, 1
            )
            for r in range(reps):
                vector.wait_ge(s_u, r * 2 + 2)
                if r > 1 and last_u2_g >= 0:
                    vector.wait_ge(s_mg, (r - 2) * n_g + last_u2_g + 1)
                if FULLSEMS and r > 1 and last_u2_d >= 0:
                    vector.wait_ge(
                        s_md, (r - 2) * dmul_per_rep + 1 + last_u2_d + 1
                    )
                nc.vector.tensor_tensor(
                    U2s[r % 2].ap(), Us[r % 2].ap(), Us[r % 2].ap(),
                    op=alu.mult,
                ).then_inc(s_md, 1)
                seen_seg = set()
                for i, t in enumerate(d_tiles):
                    gi = r * n_d + i
                    if t["seg"] not in seen_seg and t["j"] <= 2:
                        seen_seg.add(t["seg"])
                        vector.wait_ge(s_anc, r * n_act + t["seg"] + 1)
                    elif FULLSEMS and t["j"] <= 2:
                        vector.wait_ge(s_anc, r * n_act + t["seg"] + 1)
                    if FULLSEMS and t["j"] >= 2:
                        vector.wait_ge(s_md, r * dmul_per_rep + 1)
                    if FULLSEMS and t["j"] > 2:
                        prod = next(
                            k for k, q in enumerate(d_tiles)
                            if q["seg"] == t["seg"] and q["j"] == t["j"] - 2
                        )
                        vector.wait_ge(s_md, r * dmul_per_rep + 1 + prod + 1)
                    if gi >= ND and (
                        FULLSEMS or (gi - ND) % RING_BATCH_D == 0
                    ):
                        cover = min(
                            gi - ND + (1 if FULLSEMS else RING_BATCH_D) - 1,
                            reps * n_d - 1,
                        )
                        old = next(
                            q for q in pe_order
                            if q["eng"] == "D" and q["local"] == cover % n_d
                        )
                        vector.wait_ge(
                            s_pd, (cover // n_d) * n_d + old["pe_cum"]
                        )
                    if t["j"] <= 2:
                        src = Anc[(r * n_seg + t["seg"]) % NA].ap()
                    else:
                        prod = next(
                            k for k, q in enumerate(d_tiles)
                            if q["seg"] == t["seg"] and q["j"] == t["j"] - 2
                        )
                        src = Wd[(r * n_d + prod) % ND].ap()
                    mul = Us[r % 2].ap() if t["j"] == 1 else U2s[r % 2].ap()
                    nc.vector.tensor_tensor(
                        Wd[gi % ND].ap(), src, mul, op=alu.mult
                    ).then_inc(s_md, 1)
            # final: compact psum residual slots to SBUF for the out DMA
            vector.wait_ge(s_pd, reps * n_d)
            if n_g:
                vector.wait_ge(s_pg, reps * n_g)
            for stripe in range(3):
                nr = nregs[stripe]
                if nr == 0:
                    nc.vector.memset(
                        Rs.ap()[32 * stripe : 32 * stripe + 8, 0:1],
                        0.0,
                    ).then_inc(s_cp, 1)
                    continue
                nc.vector.tensor_copy(
                    Rs.ap()[32 * stripe : 32 * stripe + 8, : nr * 512],
                    ps.ap()[32 * stripe : 32 * stripe + 8, : nr * 512],
                ).then_inc(s_cp, 1)

        # ---------------- PE: block-ones reduction into PSUM slots -------
        @block.tensor
        def _(tensor):
            tensor.wait_ge(s_ones, 1)
            for r in range(reps):
                for pi, t in enumerate(pe_order):
                    slot = t["slot"]
                    # producer progress, batched over PE_BATCH tiles
                    if FULLSEMS or pi % PE_BATCH == 0:
                        grp = pe_order[
                            pi : pi + (1 if FULLSEMS else PE_BATCH)
                        ]
                        need_d = max(
                            [q["local"] for q in grp if q["eng"] == "D"],
                            default=-1,
                        )
                        need_g = max(
                            [q["local"] for q in grp if q["eng"] == "G"],
                            default=-1,
                        )
                        if need_d >= 0:
                            tensor.wait_ge(
                                s_md, r * dmul_per_rep + 1 + need_d + 1
                            )
                        if need_g >= 0:
                            tensor.wait_ge(s_mg, r * n_g + need_g + 1)
                    if FULLSEMS and r > 0:
                        # cross-rep psum WAW self-edge (auto-true on HW)
                        if t["eng"] == "D":
                            tensor.wait_ge(
                                s_pd, (r - 1) * n_d + t["pe_cum"]
                            )
                        else:
                            tensor.wait_ge(
                                s_pg, (r - 1) * n_g + t["pe_cum"]
                            )
                    bp, fo = slot_addr(slot)
                    if t["eng"] == "D":
                        w = Wd[(r * n_d + t["local"]) % ND].ap()
                    else:
                        w = Wg[(r * n_g + t["local"]) % NG].ap()
                    for q in range(8):
                        mm = nc.tensor.matmul(
                            ps.ap()[bp : bp + 8, fo : fo + 512],
                            onesb.ap(),
                            w[:, q * 512 : (q + 1) * 512],
                            start=(q == 0), stop=(q == 7),
                        )
                    if t["eng"] == "D":
                        mm.then_inc(s_pd, 1)
                    else:
                        mm.then_inc(s_pg, 1)

    _nc_cache[key] = nc
    return nc


def _build_w(bin_centers=None) -> np.ndarray:
    if bin_centers is None:
        bin_centers = np.linspace(0.0, 1.0, NBINS)
    bc = np.asarray(bin_centers, np.float64)
    w = np.zeros((128, 24), np.float32)
    act_bins = [k0 for k0, _ in SEGMENTS] + list(DIRECT)
    for i, k in enumerate(act_bins):
        w[:, i] = np.float32(-SQRT_A * bc[k])
    delta = (bc[-1] - bc[0]) / (NBINS - 1)
    w[:, 9] = np.float32(-2.0 * A_COEF * delta * GAMMA)
    for c in range(C):
        w[c * G : (c + 1) * G, 16 + c] = 1.0
    return w


def _host_combine(acta: np.ndarray, outp: np.ndarray, bc: np.ndarray) -> np.ndarray:
    """acta [128, n_act]; outp [24, 1024] (psum residuals) -> [C, NBINS]."""
    tiles, _d, _g, pe_order, _nd, _ng = _plan()
    drift, _su, _bu, _delta = _drift(bc)
    out = np.zeros((C, NBINS), np.float64)
    scale = (ER / RATIO) * (math.sqrt(math.pi) / 2.0)
    act_bins = [k0 for k0, _ in SEGMENTS] + list(DIRECT)
    a = acta.reshape(C, G, -1).sum(axis=1)
    for i, k in enumerate(act_bins):
        out[:, k] = a[:, i] * scale
    for t in pe_order:
        k = t["bin"]
        s = t["slot"]
        stripe, region = s % 3, s // 3
        vals = outp[stripe * 8 : stripe * 8 + C,
                    region * 512 : (region + 1) * 512].sum(axis=1)
        out[:, k] = vals * scale * math.exp(-drift[k])
    return out.astype(np.float32)


def kernel(x: np.ndarray, bin_centers: np.ndarray) -> np.ndarray:
    global last_results
    x = np.ascontiguousarray(np.asarray(x), dtype=np.float32)
    bc = np.asarray(bin_centers, np.float64)
    assert x.shape == (B, C, 256, 256), x.shape
    assert bc.shape == (NBINS,), bc.shape

    nc = _build(bc)
    w = _build_w(bc)
    in_maps = [{"x": x[b].reshape(C, HW), "w": w} for b in range(B)]
    res = run_bass_kernel_spmd(nc, in_maps, list(range(B)))
    last_results = res
    outs = []
    for b in range(B):
        acta = np.asarray(res.results[b]["out_a"], np.float64)
        outp = np.asarray(res.results[b]["out_p"], np.float64)
        outs.append(_host_combine(acta, outp, bc))
    return np.stack(outs).reshape(B, C * NBINS, 1, 1).astype(np.float32)
